# revision 49
# baseline (speedup 1.0000x reference)
"""GAT (2-layer, 4-head) distributed Bass kernel for Trainium2, 8 NeuronCores.

Strategy (1D node partition, dst-owner edge routing), v2:
  - Core c owns nodes [c*NLOC, (c+1)*NLOC), padded to NLOCP = T*128.
  - Per layer: each core computes feat/el/er for its own nodes via PE matmuls
    (feat = x @ W, el = x @ (W@al), er = x @ (W@ar)), writes a bf16
    [NLOCP, 256] "fel" table ([feat(128) | el(4) | pad], 512 B rows) and
    AllGathers it across the 8 cores. er stays on-chip in SBUF (only the
    dst owner needs it).
  - Edges are grouped by destination owner, then by 128-row destination tile,
    then split by source-table half (A/B) so dma_gather's int16 indices stay
    in range; each half is padded to whole 128-edge chunks, chunk counts
    maxed across cores so the SPMD IR is identical on all 8 cores.
  - The per-chunk one-hot matrices O[e, r] = (dst_row[e] == r) and their
    transposes are STATIC (host-known): they are precomputed on the host in
    bf16 and streamed in per tile as sequential DMA ("OOT" = [O | OT] blocks),
    replacing the per-chunk DVE is_equal build of v1 (which was DVE-bound)
    and the er dma_gather of v1 (desc/bandwidth-bound):
      * er_edge = OT.T-free matmul: er_ps[e, h] = sum_r OT[r, e] * er_tile[r, h]
      * s = exp(leakyrelu(el[src] + er_ps))      (DVE + ACT)
      * featw = feat * s (head-broadcast); s into 4 denominator columns
      * PSUM accumulate: agg[r, :] += O.T @ featw  (numerator | denominator)
    Pad slots have all-zero one-hot columns, so they contribute nothing.
  - Per dst tile epilogue: rst = num/max(den,1e-9) + residual (+bias);
    layer 1 applies ELU, transposes h and immediately runs the layer-2
    node matmuls for that tile (pipelined pre-phase), so only the second
    AllGather sits between the two edge phases.
  - The two feature gathers per tile rotate across 4 SWDGE queues so their
    HBM transfers overlap.

Single-pass softmax: alpha = exp(e)/sum(exp(e)) == reference's
exp(e-emax)/sum(exp(e-emax)); logits are O(1) so no overflow.
"""

import numpy as np
import ml_dtypes

# ---- problem constants (hardcoded; kernel.py must be self-contained) ----
N = 50000
E = 800000
P = 8
IN = 128
HID = 32
H = 4
F = H * HID          # 128, same for both layers
OUTD = 32
NEG = 0.2
TILE = 128

NLOC = N // P        # 6250
T = (NLOC + TILE - 1) // TILE          # 49
NLOCP = T * TILE     # 6272

ROWW = 256           # fel table row width in bf16 elems (512 B)
TLO = 31             # node tiles in the "lo" half-table (AllGathered early;
                     # 31 is the int16 max: 8*31*128 = 31744 < 32768)
THI = T - TLO        # 24 tiles in the "hi" half-table
NLO = TLO * TILE     # 3200 rows per core
NHI = THI * TILE     # 3072
BF16 = ml_dtypes.bfloat16


def _wrap16(idx):
    """[n] index list -> [128, n//16] int16, wrapped in 16 partitions and
    replicated across the 8 Q7 cores (dma_gather layout)."""
    a = np.asarray(idx).reshape(-1, 16).T
    return np.tile(a, (8, 1)).astype(np.int16)


# ----------------------------------------------------------------------------
# Host-side preprocessing
# ----------------------------------------------------------------------------

def prep_edges(src, dst, n=N, p=P):
    """Group edges by (dst owner, dst tile, src-half), pad each (core,tile,
    half) to common chunk counts KA_t/KB_t, and emit per-core index arrays.

    Returns (KAs, KBs, per_core): per_core[c] has
      gA   int16 [128, 8*sumKA]  wrapped fel-gather idxs into the lo table
      gB   int16 [128, 8*sumKB]  wrapped fel-gather idxs into the hi table
      oot  int8  [128, 2*sumK*128]  per-chunk one-hot blocks [O | OT] per tile
    """
    nloc = n // p
    t_tiles = (nloc + TILE - 1) // TILE

    owner = dst // nloc
    loc = dst - owner * nloc
    tl = loc // TILE
    row = loc - tl * TILE

    sowner = src // nloc
    sloc = src - sowner * nloc
    hb = (sloc >= NLO).astype(np.int64)           # 0 = lo table, 1 = hi
    pgid = np.where(hb == 0, sowner * NLO + sloc,
                    sowner * NHI + (sloc - NLO))

    order = np.lexsort((hb, tl, owner))
    owner_s = owner[order]
    tl_s = tl[order]
    hb_s = hb[order]
    row_s = row[order].astype(np.int64)
    pgid_s = pgid[order].astype(np.int64)

    counts = np.zeros((p, t_tiles, 2), dtype=np.int64)
    np.add.at(counts, (owner_s, tl_s, hb_s), 1)
    KAs = (-(-counts[:, :, 0] // TILE)).max(axis=0)
    KBs = (-(-counts[:, :, 1] // TILE)).max(axis=0)
    KAs = np.maximum(KAs, (KAs + KBs) == 0)       # ensure >=1 chunk per tile
    Ks = KAs + KBs
    off = np.concatenate([[0], np.cumsum(Ks)]).astype(int)
    offA = np.concatenate([[0], np.cumsum(KAs)]).astype(int)
    offB = np.concatenate([[0], np.cumsum(KBs)]).astype(int)
    sumK, sumKA, sumKB = int(off[-1]), int(offA[-1]), int(offB[-1])

    grp = (owner_s * t_tiles + tl_s) * 2 + hb_s
    gcnt = np.bincount(grp, minlength=p * t_tiles * 2)
    gstart = np.concatenate([[0], np.cumsum(gcnt)])
    within = np.arange(len(src)) - gstart[grp]
    k = within // TILE
    prt = within - k * TILE
    # chunk column in the full per-tile layout (A chunks first, then B)
    col = off[tl_s] + np.where(hb_s == 0, k, KAs[tl_s] + k)

    rng = np.arange(TILE, dtype=np.int64)
    per_core = []
    for c in range(p):
        m = owner_s == c
        dstrow = np.full((TILE, sumK), -1, dtype=np.int64)
        dstrow[prt[m], col[m]] = row_s[m]
        gfull = np.zeros((TILE, sumK), dtype=np.int64)
        gfull[prt[m], col[m]] = pgid_s[m]

        # one-hot blocks: oh[p, col, r] = (dstrow[p, col] == r)
        oh = (dstrow[:, :, None] == rng[None, None, :]).astype(np.int8)
        o8 = np.ascontiguousarray(oh.reshape(TILE, sumK * TILE))
        ohT = np.empty((TILE, sumK * TILE), dtype=ml_dtypes.float8_e4m3)
        for t in range(t_tiles):
            kt = int(Ks[t])
            o = int(off[t])
            blkT = oh[:, o:o + kt, :].transpose(2, 1, 0)  # [r, kt, p]
            ohT[:, o * TILE:(o + kt) * TILE] = \
                blkT.reshape(TILE, kt * TILE).astype(ml_dtypes.float8_e4m3)

        # flatten chunk cols into wrapped idx streams
        gA = np.zeros((TILE, 8 * sumKA), dtype=np.int16)
        gB = np.zeros((TILE, 8 * sumKB), dtype=np.int16)
        for t in range(t_tiles):
            ka, kb = int(KAs[t]), int(KBs[t])
            o, oa, ob = off[t], offA[t], offB[t]
            if ka:
                ia = gfull[:, o:o + ka].T.reshape(-1)          # i = k*128+p
                gA[:, 8 * oa:8 * (oa + ka)] = _wrap16(ia)
            if kb:
                ib = gfull[:, o + ka:o + ka + kb].T.reshape(-1)
                gB[:, 8 * ob:8 * (ob + kb)] = _wrap16(ib)
        per_core.append(dict(gA=gA, gB=gB, o8=o8, ot8=ohT))
    return [int(x) for x in KAs], [int(x) for x in KBs], per_core


def prep_weights(W, al, ar):
    """[W | W@al per head | W@ar per head] -> [in, F+2H] float32."""
    Wr = W.reshape(W.shape[0], H, -1)
    wal = np.einsum('ihd,hd->ih', Wr, al)
    war = np.einsum('ihd,hd->ih', Wr, ar)
    return np.concatenate([W, wal, war], axis=1).astype(np.float32)


def prep_node_inputs(x, b1, n=N, p=P):
    """Per-core xT ([IN, NLOCP], lhsT layout) and xb ([128, T*IN],
    tile-row-major residual layout, bias prefolded)."""
    nloc = n // p
    t_tiles = (nloc + TILE - 1) // TILE
    nlocp = t_tiles * TILE
    outs = []
    for c in range(p):
        xl = np.zeros((nlocp, x.shape[1]), dtype=np.float32)
        xl[:nloc] = x[c * nloc:(c + 1) * nloc]
        xT = np.ascontiguousarray(xl.T)
        xb = (xl + b1[None, :]).reshape(t_tiles, TILE, -1).transpose(1, 0, 2)
        xb = np.ascontiguousarray(xb.reshape(TILE, -1))
        outs.append((xT, xb))
    return outs


# ----------------------------------------------------------------------------
# Bass IR builder
# ----------------------------------------------------------------------------

def build_gat(KAs, KBs, n=N, p=P, in_dim=IN):
    import concourse.bass as bass
    import concourse.bacc as bacc
    import concourse.mybir as mybir
    import concourse.tile as tile

    f32 = mybir.dt.float32
    bf16 = mybir.dt.bfloat16
    i16 = mybir.dt.int16
    AF = mybir.ActivationFunctionType
    ALU = mybir.AluOpType

    nloc = n // p
    t_tiles = (nloc + TILE - 1) // TILE
    nlocp = t_tiles * TILE
    KAs = list(KAs)
    KBs = list(KBs)
    Ks = [a + b for a, b in zip(KAs, KBs)]
    off = np.concatenate([[0], np.cumsum(Ks)]).astype(int)
    offA = np.concatenate([[0], np.cumsum(KAs)]).astype(int)
    offB = np.concatenate([[0], np.cumsum(KBs)]).astype(int)
    sumK, sumKA, sumKB = int(off[-1]), int(offA[-1]), int(offB[-1])
    Kmax = max(Ks)
    rg = [list(range(p))]

    nc = bacc.Bacc("TRN2", target_bir_lowering=False, num_swdge_queues=4)

    # ---- I/O ----
    xT_in = nc.dram_tensor("xT", [in_dim, nlocp], f32, kind="ExternalInput")
    xb_in = nc.dram_tensor("xb", [TILE, t_tiles * in_dim], f32, kind="ExternalInput")
    W1_in = nc.dram_tensor("Wcat1", [in_dim, F + 2 * H], f32, kind="ExternalInput")
    W2_in = nc.dram_tensor("Wcat2", [F, F + 2 * H], f32, kind="ExternalInput")
    b2r_in = nc.dram_tensor("b2r", [TILE, F], f32, kind="ExternalInput")
    ident_in = nc.dram_tensor("ident", [TILE, TILE], f32, kind="ExternalInput")
    gA_in = nc.dram_tensor("gA", [TILE, 8 * sumKA], i16, kind="ExternalInput")
    gB_in = nc.dram_tensor("gB", [TILE, max(8 * sumKB, 16)], i16, kind="ExternalInput")
    o8_in = nc.dram_tensor("O8", [TILE, sumK * TILE], mybir.dt.int8,
                           kind="ExternalInput")
    ot8_in = nc.dram_tensor("OT8", [TILE, sumK * TILE], mybir.dt.float8e4,
                            kind="ExternalInput")
    out_ext = nc.dram_tensor("out", [nlocp, OUTD], f32, kind="ExternalOutput")

    # ---- internal DRAM ----
    fel_loc = [[nc.dram_tensor(f"fel_loc{i}{h}", [nn, ROWW], bf16)
                for h, nn in (("lo", NLO), ("hi", NHI))] for i in (1, 2)]
    fel_full = [[nc.dram_tensor(f"fel_full{i}{h}", [p * nn, ROWW], bf16,
                                addr_space="Shared")
                 for h, nn in (("lo", NLO), ("hi", NHI))] for i in (1, 2)]

    with tile.TileContext(nc) as tc:
        with tc.tile_pool(name="cst", bufs=1) as cst, \
             tc.tile_pool(name="big", bufs=1) as big, \
             tc.tile_pool(name="fe", bufs=5) as fep, \
             tc.tile_pool(name="oo", bufs=3) as oop, \
             tc.tile_pool(name="xbp", bufs=3) as xbp, \
             tc.tile_pool(name="wk", bufs=6) as wk, \
             tc.tile_pool(name="ep", bufs=3) as ep, \
             tc.tile_pool(name="ps", bufs=1, space="PSUM") as ps:

            xT = cst.sbuf_tile_from(xT_in.ap())
            Wc1 = cst.sbuf_tile_from(W1_in.ap())
            Wc2 = cst.sbuf_tile_from(W2_in.ap())
            b2r = cst.sbuf_tile_from(b2r_in.ap())
            ident = cst.sbuf_tile_from(ident_in.ap())
            gA = cst.sbuf_tile_from(gA_in.ap())
            gB = cst.sbuf_tile_from(gB_in.ap())

            h_sb = big.tile([TILE, t_tiles * F], f32)
            # per-node er for both layers, fp8 (rhs of the fp8 er matmul)
            er_sb = big.tile([TILE, 2 * t_tiles * H], mybir.dt.float8e4)

            def pre_tile(lhsT_ap, Wc, layer, nt):
                """node matmuls for one 128-node tile -> fel_loc rows + er_sb."""
                if nt < TLO:
                    dst_t = fel_loc[layer][0]
                    sl = slice(nt * TILE, (nt + 1) * TILE)
                else:
                    dst_t = fel_loc[layer][1]
                    sl = slice((nt - TLO) * TILE, (nt - TLO + 1) * TILE)
                pf = ps.tile([TILE, F], f32, tag="pf", bufs=2, name=f"pf{layer}_{nt}")
                nc.tensor.matmul(pf[:, :], lhsT=lhsT_ap, rhs=Wc[:, 0:F],
                                 start=True, stop=True)
                p8 = ps.tile([TILE, 2 * H], f32, tag="p8", bufs=1, name=f"p8{layer}_{nt}")
                nc.tensor.matmul(p8[:, :], lhsT=lhsT_ap, rhs=Wc[:, F:F + 2 * H],
                                 start=True, stop=True)
                fel = ep.tile([TILE, ROWW], bf16, tag="fel", name=f"fel{layer}_{nt}")
                nc.vector.tensor_copy(fel[:, 0:F], pf[:, :])
                nc.vector.tensor_copy(fel[:, F:F + H], p8[:, 0:H])
                nc.vector.tensor_copy(er_sb[:, (layer * t_tiles + nt) * H:
                                             (layer * t_tiles + nt + 1) * H],
                                      p8[:, H:2 * H])
                nc.sync.dma_start(dst_t[sl, 0:F + H], fel[:, 0:F + H])

            def allgather(layer, part):
                nc.gpsimd.collective_compute(
                    "AllGather", mybir.AluOpType.bypass, replica_groups=rg,
                    ins=[fel_loc[layer][part].ap().opt()],
                    outs=[fel_full[layer][part].ap().opt()])

            def stage_a(layer, t):
                """gathers + one-hot loads + er matmuls for dst tile t."""
                ka, kb = KAs[t], KBs[t]
                kt = ka + kb
                o0, oa, ob = int(off[t]), int(offA[t]), int(offB[t])
                o8t = oop.tile([TILE, kt * TILE], mybir.dt.int8, tag="o8",
                               bufs=2, padded_shape=[TILE, Kmax * TILE],
                               name=f"o8{layer}_{t}")
                nc.scalar.dma_start(o8t[:, :],
                                    o8_in[:, o0 * TILE:(o0 + kt) * TILE])
                obf = oop.tile([TILE, kt * TILE], bf16, tag="obf",
                               padded_shape=[TILE, Kmax * TILE],
                               name=f"obf{layer}_{t}")
                nc.vector.tensor_copy(obf[:, :], o8t[:, :])
                ot8 = oop.tile([TILE, kt * TILE], mybir.dt.float8e4, tag="ot8",
                               padded_shape=[TILE, Kmax * TILE],
                               name=f"ot8{layer}_{t}")
                nc.scalar.dma_start(ot8[:, :],
                                    ot8_in[:, o0 * TILE:(o0 + kt) * TILE])
                fe = fep.tile([TILE, kt, ROWW], bf16, tag="fe",
                              padded_shape=[TILE, Kmax, ROWW], name=f"fe{layer}_{t}")
                # balance the tile's kt chunks evenly across the 4 SWDGE
                # queues; a queue's range may span the lo/hi table boundary
                # (then it becomes two gather calls)
                bounds = [(i * kt + 2) // 4 for i in range(5)]
                for q in range(4):
                    s, e = bounds[q], bounds[q + 1]
                    if s < min(e, ka):
                        lo, hi = s, min(e, ka)
                        nc.gpsimd.dma_gather(
                            fe[:, lo:hi, :], fel_full[layer][0].ap(),
                            gA[:, 8 * (oa + lo):8 * (oa + hi)],
                            (hi - lo) * TILE, (hi - lo) * TILE, ROWW,
                            single_packet=True, queue_num=q)
                    if e > max(s, ka):
                        lo, hi = max(s, ka) - ka, e - ka
                        nc.gpsimd.dma_gather(
                            fe[:, ka + lo:ka + hi, :], fel_full[layer][1].ap(),
                            gB[:, 8 * (ob + lo):8 * (ob + hi)],
                            (hi - lo) * TILE, (hi - lo) * TILE, ROWW,
                            single_packet=True, queue_num=q)
                er_ps = ps.tile([TILE, Kmax * H], f32, tag="er", bufs=2,
                                name=f"erps{layer}_{t}")
                ert = er_sb[:, (layer * t_tiles + t) * H:(layer * t_tiles + t + 1) * H]
                for k in range(kt):
                    nc.tensor.matmul(er_ps[:, k * H:(k + 1) * H],
                                     lhsT=ot8[:, k * TILE:(k + 1) * TILE],
                                     rhs=ert, start=True, stop=True)
                return fe, obf, er_ps

            def stage_b(layer, t, fe, obf, er_ps):
                """SDDMM + softmax-weighted aggregation for dst tile t."""
                ka, kb = KAs[t], KBs[t]
                kt = ka + kb
                lg = wk.tile([TILE, kt * H], f32, tag="lg", bufs=3,
                             padded_shape=[TILE, Kmax * H], name=f"lg{layer}_{t}")
                nc.vector.tensor_tensor(lg[:, :], fe[:, :, F:F + H],
                                        er_ps[:, 0:kt * H], op=ALU.add)
                lr = wk.tile([TILE, kt * H], f32, tag="lr", bufs=3,
                             padded_shape=[TILE, Kmax * H], name=f"lr{layer}_{t}")
                nc.vector.scalar_tensor_tensor(lr[:, :], lg[:, :], NEG, lg[:, :],
                                               ALU.mult, ALU.max)
                fw = wk.tile([TILE, kt, F + H], bf16, tag="fw", bufs=3,
                             padded_shape=[TILE, Kmax, F + H], name=f"fw{layer}_{t}")
                nc.scalar.activation(fw[:, :, F:F + H], lr[:, :], AF.Exp)
                sv = fw[:, :, F:F + H]
                s_b = bass.AP(sv.tensor, sv.offset,
                              [sv.ap[0], [F + H, kt], [1, H], [0, HID]])
                nc.vector.tensor_tensor(fw[:, :, 0:F], fe[:, :, 0:F], s_b,
                                        op=ALU.mult)
                agg = ps.tile([TILE, F + H], f32, tag="agg", bufs=2,
                              name=f"agg{layer}_{t}")
                for k in range(kt):
                    nc.tensor.matmul(agg[:, :], lhsT=obf[:, k * TILE:(k + 1) * TILE],
                                     rhs=fw[:, k, :],
                                     start=(k == 0), stop=(k == kt - 1))
                # ---- epilogue ----
                sl128 = slice(t * TILE, (t + 1) * TILE)
                slF = slice(t * F, (t + 1) * F)
                den = wk.tile([TILE, H], f32, tag="den", name=f"den{layer}_{t}")
                nc.vector.tensor_scalar(den[:, :], agg[:, F:F + H], 1e-9, None,
                                        op0=ALU.max)
                rec = wk.tile([TILE, H], f32, tag="rec", name=f"rec{layer}_{t}")
                nc.vector.reciprocal(rec[:, :], den[:, :])
                rst = ep.tile([TILE, F], f32, tag="rst", name=f"rst{layer}_{t}")
                av = agg[:, 0:F]
                a_b = bass.AP(av.tensor, av.offset, [av.ap[0], [HID, H], [1, HID]])
                rv = rec[:, 0:H]
                r_b = bass.AP(rv.tensor, rv.offset, [rv.ap[0], [1, H], [0, HID]])
                ov = rst[:, 0:F]
                o_b = bass.AP(ov.tensor, ov.offset, [ov.ap[0], [HID, H], [1, HID]])
                nc.vector.tensor_tensor(o_b, a_b, r_b, op=ALU.mult)
                if layer == 0:
                    xb_t = xbp.tile([TILE, F], f32, tag="xb", name=f"xb_{t}")
                    nc.scalar.dma_start(xb_t[:, :], xb_in[:, slF])
                    nc.vector.tensor_tensor(rst[:, :], rst[:, :], xb_t[:, :],
                                            op=ALU.add)
                    # ELU -> h
                    r1 = ep.tile([TILE, F], f32, tag="r1", name=f"r1_{t}")
                    nc.scalar.activation(r1[:, :], rst[:, :], AF.Relu)
                    r2 = ep.tile([TILE, F], f32, tag="r2", name=f"r2_{t}")
                    nc.scalar.activation(r2[:, :], rst[:, :], AF.Relu, scale=-1.0)
                    r3 = ep.tile([TILE, F], f32, tag="r3", name=f"r3_{t}")
                    nc.scalar.activation(r3[:, :], r2[:, :], AF.Exp, scale=-1.0)
                    nc.vector.scalar_tensor_tensor(h_sb[:, slF], r3[:, :], -1.0,
                                                   r1[:, :], ALU.add, ALU.add)
                    ptr = ps.tile([TILE, TILE], f32, tag="tr", bufs=1, name=f"tr_{t}")
                    nc.tensor.transpose(ptr[:, :], h_sb[:, slF], ident[:, :])
                    ht = ep.tile([TILE, TILE], f32, tag="ht", name=f"ht_{t}")
                    nc.vector.tensor_copy(ht[:, :], ptr[:, :])
                    # pipelined layer-2 node matmuls for this tile
                    pre_tile(ht[:, :], Wc2, 1, t)
                else:
                    nc.vector.tensor_tensor(rst[:, :], rst[:, :], h_sb[:, slF],
                                            op=ALU.add)
                    nc.vector.tensor_tensor(rst[:, :], rst[:, :], b2r[:, :],
                                            op=ALU.add)
                    m1 = ep.tile([TILE, OUTD], f32, tag="m1", name=f"m1_{t}")
                    nc.vector.tensor_tensor(m1[:, :], rst[:, 0:OUTD],
                                            rst[:, OUTD:2 * OUTD], op=ALU.add)
                    m2 = ep.tile([TILE, OUTD], f32, tag="m2", name=f"m2_{t}")
                    nc.vector.tensor_tensor(m2[:, :], rst[:, 2 * OUTD:3 * OUTD],
                                            rst[:, 3 * OUTD:4 * OUTD], op=ALU.add)
                    ot = ep.tile([TILE, OUTD], f32, tag="ot", name=f"ot_{t}")
                    nc.vector.tensor_tensor(ot[:, :], m1[:, :], m2[:, :], op=ALU.add)
                    of = ep.tile([TILE, OUTD], f32, tag="of", name=f"of_{t}")
                    nc.vector.tensor_scalar(of[:, :], ot[:, :], 0.25, None,
                                            op0=ALU.mult)
                    nc.sync.dma_start(out_ext[t * TILE:(t + 1) * TILE, :], of[:, :])

            def edge_phase(layer, post_b=None, skew=1):
                pend = []
                for t in range(t_tiles):
                    pend.append(stage_a(layer, t))
                    if t >= skew:
                        stage_b(layer, t - skew, *pend[t - skew])
                        if post_b and (t - skew) in post_b:
                            post_b[t - skew]()
                for t in range(t_tiles - skew, t_tiles):
                    stage_b(layer, t, *pend[t])
                    if post_b and t in post_b:
                        post_b[t]()

            # ================= layer 1 =================
            for nt in range(t_tiles):
                pre_tile(xT[:, nt * TILE:(nt + 1) * TILE], Wc1, 0, nt)
                if nt == TLO - 1:
                    allgather(0, 0)      # lo half ships while hi computes
            allgather(0, 1)
            # layer-2 fel halves ship as soon as their epilogues finish
            edge_phase(0, post_b={TLO - 1: lambda: allgather(1, 0),
                                  t_tiles - 1: lambda: allgather(1, 1)})
            # ================= layer 2 =================
            edge_phase(1)

    nc.compile()
    return nc


# ----------------------------------------------------------------------------
# Host entry point
# ----------------------------------------------------------------------------

def make_inputs(x, W1, al1, ar1, b1, W2, al2, ar2, b2, src, dst, n=N, p=P):
    KAs, KBs, per_core = prep_edges(np.asarray(src), np.asarray(dst), n=n, p=p)
    Wcat1 = prep_weights(np.asarray(W1, np.float32), np.asarray(al1, np.float32),
                         np.asarray(ar1, np.float32))
    Wcat2 = prep_weights(np.asarray(W2, np.float32), np.asarray(al2, np.float32),
                         np.asarray(ar2, np.float32))
    node_in = prep_node_inputs(np.asarray(x, np.float32), np.asarray(b1, np.float32),
                               n=n, p=p)
    b2r = np.tile(np.asarray(b2, np.float32)[None, :], (TILE, 1))
    ident = np.eye(TILE, dtype=np.float32)
    in_maps = []
    for c in range(p):
        xT, xb = node_in[c]
        pc = per_core[c]
        gB = pc["gB"] if pc["gB"].shape[1] else np.zeros((TILE, 16), np.int16)
        in_maps.append(dict(
            xT=xT, xb=xb, Wcat1=Wcat1, Wcat2=Wcat2, b2r=b2r, ident=ident,
            gA=pc["gA"], gB=gB, O8=pc["o8"], OT8=pc["ot8"]))
    return KAs, KBs, in_maps


def kernel(x, W1, al1, ar1, b1, W2, al2, ar2, b2, src, dst, **run_kwargs):
    from concourse.bass_utils import run_bass_kernel_spmd
    KAs, KBs, in_maps = make_inputs(x, W1, al1, ar1, b1, W2, al2, ar2, b2, src, dst)
    nc = build_gat(KAs, KBs)
    res = run_bass_kernel_spmd(nc, in_maps, core_ids=list(range(P)), **run_kwargs)
    out = np.concatenate([r["out"][:NLOC] for r in res.results], axis=0)
    if run_kwargs:
        return out.astype(np.float32), res
    return out.astype(np.float32)


# revision 50
# speedup vs baseline: 1.1019x; 1.1019x over previous
"""GAT (2-layer, 4-head) distributed Bass kernel for Trainium2, 8 NeuronCores.

Strategy (1D node partition, dst-owner edge routing), v2:
  - Core c owns nodes [c*NLOC, (c+1)*NLOC), padded to NLOCP = T*128.
  - Per layer: each core computes feat/el/er for its own nodes via PE matmuls
    (feat = x @ W, el = x @ (W@al), er = x @ (W@ar)), writes a bf16
    [NLOCP, 256] "fel" table ([feat(128) | el(4) | pad], 512 B rows) and
    AllGathers it across the 8 cores. er stays on-chip in SBUF (only the
    dst owner needs it).
  - Edges are grouped by destination owner, then by 128-row destination tile,
    then split by source-table half (A/B) so dma_gather's int16 indices stay
    in range; each half is padded to whole 128-edge chunks, chunk counts
    maxed across cores so the SPMD IR is identical on all 8 cores.
  - The per-chunk one-hot matrices O[e, r] = (dst_row[e] == r) and their
    transposes are STATIC (host-known): they are precomputed on the host in
    bf16 and streamed in per tile as sequential DMA ("OOT" = [O | OT] blocks),
    replacing the per-chunk DVE is_equal build of v1 (which was DVE-bound)
    and the er dma_gather of v1 (desc/bandwidth-bound):
      * er_edge = OT.T-free matmul: er_ps[e, h] = sum_r OT[r, e] * er_tile[r, h]
      * s = exp(leakyrelu(el[src] + er_ps))      (DVE + ACT)
      * featw = feat * s (head-broadcast); s into 4 denominator columns
      * PSUM accumulate: agg[r, :] += O.T @ featw  (numerator | denominator)
    Pad slots have all-zero one-hot columns, so they contribute nothing.
  - Per dst tile epilogue: rst = num/max(den,1e-9) + residual (+bias);
    layer 1 applies ELU, transposes h and immediately runs the layer-2
    node matmuls for that tile (pipelined pre-phase), so only the second
    AllGather sits between the two edge phases.
  - The two feature gathers per tile rotate across 4 SWDGE queues so their
    HBM transfers overlap.

Single-pass softmax: alpha = exp(e)/sum(exp(e)) == reference's
exp(e-emax)/sum(exp(e-emax)); logits are O(1) so no overflow.
"""

import numpy as np
import ml_dtypes

# ---- problem constants (hardcoded; kernel.py must be self-contained) ----
N = 50000
E = 800000
P = 8
IN = 128
HID = 32
H = 4
F = H * HID          # 128, same for both layers
OUTD = 32
NEG = 0.2
TILE = 128

NLOC = N // P        # 6250
T = (NLOC + TILE - 1) // TILE          # 49
NLOCP = T * TILE     # 6272

ROWW = 256           # fel table row width in bf16 elems (512 B)
TLO = 31             # node tiles in the "lo" half-table (AllGathered early;
                     # 31 is the int16 max: 8*31*128 = 31744 < 32768)
THI = T - TLO        # 24 tiles in the "hi" half-table
NLO = TLO * TILE     # 3200 rows per core
NHI = THI * TILE     # 3072
BF16 = ml_dtypes.bfloat16


def _wrap16(idx):
    """[n] index list -> [128, n//16] int16, wrapped in 16 partitions and
    replicated across the 8 Q7 cores (dma_gather layout)."""
    a = np.asarray(idx).reshape(-1, 16).T
    return np.tile(a, (8, 1)).astype(np.int16)


# ----------------------------------------------------------------------------
# Host-side preprocessing
# ----------------------------------------------------------------------------

def prep_edges(src, dst, n=N, p=P):
    """Group edges by (dst owner, dst tile, src-half), pad each (core,tile,
    half) to common chunk counts KA_t/KB_t, and emit per-core index arrays.

    Returns (KAs, KBs, per_core): per_core[c] has
      gA   int16 [128, 8*sumKA]  wrapped fel-gather idxs into the lo table
      gB   int16 [128, 8*sumKB]  wrapped fel-gather idxs into the hi table
      oot  int8  [128, 2*sumK*128]  per-chunk one-hot blocks [O | OT] per tile
    """
    nloc = n // p
    t_tiles = (nloc + TILE - 1) // TILE

    owner = dst // nloc
    loc = dst - owner * nloc
    tl = loc // TILE
    row = loc - tl * TILE

    sowner = src // nloc
    sloc = src - sowner * nloc
    hb = (sloc >= NLO).astype(np.int64)           # 0 = lo table, 1 = hi
    pgid = np.where(hb == 0, sowner * NLO + sloc,
                    sowner * NHI + (sloc - NLO))

    order = np.lexsort((hb, tl, owner))
    owner_s = owner[order]
    tl_s = tl[order]
    hb_s = hb[order]
    row_s = row[order].astype(np.int64)
    pgid_s = pgid[order].astype(np.int64)

    counts = np.zeros((p, t_tiles, 2), dtype=np.int64)
    np.add.at(counts, (owner_s, tl_s, hb_s), 1)
    KAs = (-(-counts[:, :, 0] // TILE)).max(axis=0)
    KBs = (-(-counts[:, :, 1] // TILE)).max(axis=0)
    KAs = np.maximum(KAs, (KAs + KBs) == 0)       # ensure >=1 chunk per tile
    Ks = KAs + KBs
    off = np.concatenate([[0], np.cumsum(Ks)]).astype(int)
    offA = np.concatenate([[0], np.cumsum(KAs)]).astype(int)
    offB = np.concatenate([[0], np.cumsum(KBs)]).astype(int)
    sumK, sumKA, sumKB = int(off[-1]), int(offA[-1]), int(offB[-1])

    grp = (owner_s * t_tiles + tl_s) * 2 + hb_s
    gcnt = np.bincount(grp, minlength=p * t_tiles * 2)
    gstart = np.concatenate([[0], np.cumsum(gcnt)])
    within = np.arange(len(src)) - gstart[grp]
    k = within // TILE
    prt = within - k * TILE
    # chunk column in the full per-tile layout (A chunks first, then B)
    col = off[tl_s] + np.where(hb_s == 0, k, KAs[tl_s] + k)

    rng = np.arange(TILE, dtype=np.int64)
    per_core = []
    for c in range(p):
        m = owner_s == c
        dstrow = np.full((TILE, sumK), -1, dtype=np.int64)
        dstrow[prt[m], col[m]] = row_s[m]
        gfull = np.zeros((TILE, sumK), dtype=np.int64)
        gfull[prt[m], col[m]] = pgid_s[m]

        # one-hot blocks: oh[p, col, r] = (dstrow[p, col] == r)
        oh = (dstrow[:, :, None] == rng[None, None, :]).astype(np.int8)
        o8 = np.ascontiguousarray(oh.reshape(TILE, sumK * TILE))
        ohT = np.empty((TILE, sumK * TILE), dtype=ml_dtypes.float8_e4m3)
        for t in range(t_tiles):
            kt = int(Ks[t])
            o = int(off[t])
            blkT = oh[:, o:o + kt, :].transpose(2, 1, 0)  # [r, kt, p]
            ohT[:, o * TILE:(o + kt) * TILE] = \
                blkT.reshape(TILE, kt * TILE).astype(ml_dtypes.float8_e4m3)

        # flatten chunk cols into wrapped idx streams
        gA = np.zeros((TILE, 8 * sumKA), dtype=np.int16)
        gB = np.zeros((TILE, 8 * sumKB), dtype=np.int16)
        for t in range(t_tiles):
            ka, kb = int(KAs[t]), int(KBs[t])
            o, oa, ob = off[t], offA[t], offB[t]
            if ka:
                ia = gfull[:, o:o + ka].T.reshape(-1)          # i = k*128+p
                gA[:, 8 * oa:8 * (oa + ka)] = _wrap16(ia)
            if kb:
                ib = gfull[:, o + ka:o + ka + kb].T.reshape(-1)
                gB[:, 8 * ob:8 * (ob + kb)] = _wrap16(ib)
        per_core.append(dict(gA=gA, gB=gB, o8=o8, ot8=ohT))
    return [int(x) for x in KAs], [int(x) for x in KBs], per_core


def prep_weights(W, al, ar):
    """[W | W@al per head | W@ar per head] -> [in, F+2H] float32."""
    Wr = W.reshape(W.shape[0], H, -1)
    wal = np.einsum('ihd,hd->ih', Wr, al)
    war = np.einsum('ihd,hd->ih', Wr, ar)
    return np.concatenate([W, wal, war], axis=1).astype(np.float32)


def prep_node_inputs(x, b1, n=N, p=P):
    """Per-core xT ([IN, NLOCP], lhsT layout) and xb ([128, T*IN],
    tile-row-major residual layout, bias prefolded)."""
    nloc = n // p
    t_tiles = (nloc + TILE - 1) // TILE
    nlocp = t_tiles * TILE
    outs = []
    for c in range(p):
        xl = np.zeros((nlocp, x.shape[1]), dtype=np.float32)
        xl[:nloc] = x[c * nloc:(c + 1) * nloc]
        xT = np.ascontiguousarray(xl.T)
        xb = (xl + b1[None, :]).reshape(t_tiles, TILE, -1).transpose(1, 0, 2)
        xb = np.ascontiguousarray(xb.reshape(TILE, -1))
        outs.append((xT, xb))
    return outs


# ----------------------------------------------------------------------------
# Bass IR builder
# ----------------------------------------------------------------------------

def build_gat(KAs, KBs, n=N, p=P, in_dim=IN):
    import concourse.bass as bass
    import concourse.bacc as bacc
    import concourse.mybir as mybir
    import concourse.tile as tile

    f32 = mybir.dt.float32
    bf16 = mybir.dt.bfloat16
    i16 = mybir.dt.int16
    AF = mybir.ActivationFunctionType
    ALU = mybir.AluOpType

    nloc = n // p
    t_tiles = (nloc + TILE - 1) // TILE
    nlocp = t_tiles * TILE
    KAs = list(KAs)
    KBs = list(KBs)
    Ks = [a + b for a, b in zip(KAs, KBs)]
    off = np.concatenate([[0], np.cumsum(Ks)]).astype(int)
    offA = np.concatenate([[0], np.cumsum(KAs)]).astype(int)
    offB = np.concatenate([[0], np.cumsum(KBs)]).astype(int)
    sumK, sumKA, sumKB = int(off[-1]), int(offA[-1]), int(offB[-1])
    Kmax = max(Ks)
    rg = [list(range(p))]

    nc = bacc.Bacc("TRN2", target_bir_lowering=False, num_swdge_queues=4)

    # ---- I/O ----
    xT_in = nc.dram_tensor("xT", [in_dim, nlocp], f32, kind="ExternalInput")
    xb_in = nc.dram_tensor("xb", [TILE, t_tiles * in_dim], f32, kind="ExternalInput")
    W1_in = nc.dram_tensor("Wcat1", [in_dim, F + 2 * H], f32, kind="ExternalInput")
    W2_in = nc.dram_tensor("Wcat2", [F, F + 2 * H], f32, kind="ExternalInput")
    b2r_in = nc.dram_tensor("b2r", [TILE, F], f32, kind="ExternalInput")
    ident_in = nc.dram_tensor("ident", [TILE, TILE], f32, kind="ExternalInput")
    gA_in = nc.dram_tensor("gA", [TILE, 8 * sumKA], i16, kind="ExternalInput")
    gB_in = nc.dram_tensor("gB", [TILE, max(8 * sumKB, 16)], i16, kind="ExternalInput")
    o8_in = nc.dram_tensor("O8", [TILE, sumK * TILE], mybir.dt.int8,
                           kind="ExternalInput")
    ot8_in = nc.dram_tensor("OT8", [TILE, sumK * TILE], mybir.dt.float8e4,
                            kind="ExternalInput")
    out_ext = nc.dram_tensor("out", [nlocp, OUTD], f32, kind="ExternalOutput")

    # ---- internal DRAM ----
    fel_loc = [[nc.dram_tensor(f"fel_loc{i}{h}", [nn, ROWW], bf16)
                for h, nn in (("lo", NLO), ("hi", NHI))] for i in (1, 2)]
    fel_full = [[nc.dram_tensor(f"fel_full{i}{h}", [p * nn, ROWW], bf16,
                                addr_space="Shared")
                 for h, nn in (("lo", NLO), ("hi", NHI))] for i in (1, 2)]

    with tile.TileContext(nc) as tc:
        with tc.tile_pool(name="cst", bufs=1) as cst, \
             tc.tile_pool(name="big", bufs=1) as big, \
             tc.tile_pool(name="fe", bufs=4) as fep, \
             tc.tile_pool(name="oo", bufs=3) as oop, \
             tc.tile_pool(name="xbp", bufs=3) as xbp, \
             tc.tile_pool(name="wk", bufs=6) as wk, \
             tc.tile_pool(name="ep", bufs=3) as ep, \
             tc.tile_pool(name="ps", bufs=1, space="PSUM") as ps:

            xT = cst.sbuf_tile_from(xT_in.ap())
            Wc1 = cst.sbuf_tile_from(W1_in.ap())
            Wc2 = cst.sbuf_tile_from(W2_in.ap())
            b2r = cst.sbuf_tile_from(b2r_in.ap())
            ident = cst.sbuf_tile_from(ident_in.ap())
            gA = cst.sbuf_tile_from(gA_in.ap())
            gB = cst.sbuf_tile_from(gB_in.ap())

            h_sb = big.tile([TILE, t_tiles * F], f32)
            # per-node er for both layers, fp8 (rhs of the fp8 er matmul)
            er_sb = big.tile([TILE, 2 * t_tiles * H], mybir.dt.float8e4)

            def pre_tile(lhsT_ap, Wc, layer, nt):
                """node matmuls for one 128-node tile -> fel_loc rows + er_sb."""
                if nt < TLO:
                    dst_t = fel_loc[layer][0]
                    sl = slice(nt * TILE, (nt + 1) * TILE)
                else:
                    dst_t = fel_loc[layer][1]
                    sl = slice((nt - TLO) * TILE, (nt - TLO + 1) * TILE)
                pf = ps.tile([TILE, F], f32, tag="pf", bufs=2, name=f"pf{layer}_{nt}")
                nc.tensor.matmul(pf[:, :], lhsT=lhsT_ap, rhs=Wc[:, 0:F],
                                 start=True, stop=True)
                p8 = ps.tile([TILE, 2 * H], f32, tag="p8", bufs=1, name=f"p8{layer}_{nt}")
                nc.tensor.matmul(p8[:, :], lhsT=lhsT_ap, rhs=Wc[:, F:F + 2 * H],
                                 start=True, stop=True)
                fel = ep.tile([TILE, ROWW], bf16, tag="fel", name=f"fel{layer}_{nt}")
                nc.vector.tensor_copy(fel[:, 0:F], pf[:, :])
                nc.vector.tensor_copy(fel[:, F:F + H], p8[:, 0:H])
                nc.vector.tensor_copy(er_sb[:, (layer * t_tiles + nt) * H:
                                             (layer * t_tiles + nt + 1) * H],
                                      p8[:, H:2 * H])
                nc.sync.dma_start(dst_t[sl, 0:F + H], fel[:, 0:F + H])

            def allgather(layer, part):
                nc.gpsimd.collective_compute(
                    "AllGather", mybir.AluOpType.bypass, replica_groups=rg,
                    ins=[fel_loc[layer][part].ap().opt()],
                    outs=[fel_full[layer][part].ap().opt()])

            def stage_a(layer, t):
                """gathers + one-hot loads + er matmuls for dst tile t."""
                ka, kb = KAs[t], KBs[t]
                kt = ka + kb
                o0, oa, ob = int(off[t]), int(offA[t]), int(offB[t])
                o8t = oop.tile([TILE, kt * TILE], mybir.dt.int8, tag="o8",
                               bufs=2, padded_shape=[TILE, Kmax * TILE],
                               name=f"o8{layer}_{t}")
                nc.scalar.dma_start(o8t[:, :],
                                    o8_in[:, o0 * TILE:(o0 + kt) * TILE])
                obf = oop.tile([TILE, kt * TILE], bf16, tag="obf",
                               padded_shape=[TILE, Kmax * TILE],
                               name=f"obf{layer}_{t}")
                nc.vector.tensor_copy(obf[:, :], o8t[:, :])
                ot8 = oop.tile([TILE, kt * TILE], mybir.dt.float8e4, tag="ot8",
                               padded_shape=[TILE, Kmax * TILE],
                               name=f"ot8{layer}_{t}")
                nc.scalar.dma_start(ot8[:, :],
                                    ot8_in[:, o0 * TILE:(o0 + kt) * TILE])
                fe = fep.tile([TILE, kt, ROWW], bf16, tag="fe",
                              padded_shape=[TILE, Kmax, ROWW], name=f"fe{layer}_{t}")
                # balance the tile's kt chunks evenly across the 4 SWDGE
                # queues; a queue's range may span the lo/hi table boundary
                # (then it becomes two gather calls)
                bounds = [(i * kt + 2) // 4 for i in range(5)]
                for q in range(4):
                    s, e = bounds[q], bounds[q + 1]
                    if s < min(e, ka):
                        lo, hi = s, min(e, ka)
                        nc.gpsimd.dma_gather(
                            fe[:, lo:hi, :], fel_full[layer][0].ap(),
                            gA[:, 8 * (oa + lo):8 * (oa + hi)],
                            (hi - lo) * TILE, (hi - lo) * TILE, ROWW,
                            single_packet=True, queue_num=q)
                    if e > max(s, ka):
                        lo, hi = max(s, ka) - ka, e - ka
                        nc.gpsimd.dma_gather(
                            fe[:, ka + lo:ka + hi, :], fel_full[layer][1].ap(),
                            gB[:, 8 * (ob + lo):8 * (ob + hi)],
                            (hi - lo) * TILE, (hi - lo) * TILE, ROWW,
                            single_packet=True, queue_num=q)
                er_ps = ps.tile([TILE, Kmax * H], f32, tag="er", bufs=2,
                                name=f"erps{layer}_{t}")
                ert = er_sb[:, (layer * t_tiles + t) * H:(layer * t_tiles + t + 1) * H]
                for k in range(kt):
                    nc.tensor.matmul(er_ps[:, k * H:(k + 1) * H],
                                     lhsT=ot8[:, k * TILE:(k + 1) * TILE],
                                     rhs=ert, start=True, stop=True)
                return fe, obf, er_ps

            def stage_b(layer, t, fe, obf, er_ps):
                """SDDMM + softmax-weighted aggregation for dst tile t."""
                ka, kb = KAs[t], KBs[t]
                kt = ka + kb
                lg = wk.tile([TILE, kt * H], f32, tag="lg", bufs=3,
                             padded_shape=[TILE, Kmax * H], name=f"lg{layer}_{t}")
                nc.vector.tensor_tensor(lg[:, :], fe[:, :, F:F + H],
                                        er_ps[:, 0:kt * H], op=ALU.add)
                lr = wk.tile([TILE, kt * H], f32, tag="lr", bufs=3,
                             padded_shape=[TILE, Kmax * H], name=f"lr{layer}_{t}")
                nc.vector.scalar_tensor_tensor(lr[:, :], lg[:, :], NEG, lg[:, :],
                                               ALU.mult, ALU.max)
                fw = wk.tile([TILE, kt, F + H], bf16, tag="fw", bufs=3,
                             padded_shape=[TILE, Kmax, F + H], name=f"fw{layer}_{t}")
                nc.scalar.activation(fw[:, :, F:F + H], lr[:, :], AF.Exp)
                sv = fw[:, :, F:F + H]
                s_b = bass.AP(sv.tensor, sv.offset,
                              [sv.ap[0], [F + H, kt], [1, H], [0, HID]])
                nc.vector.tensor_tensor(fw[:, :, 0:F], fe[:, :, 0:F], s_b,
                                        op=ALU.mult)
                agg = ps.tile([TILE, F + H], f32, tag="agg", bufs=2,
                              name=f"agg{layer}_{t}")
                for k in range(kt):
                    nc.tensor.matmul(agg[:, :], lhsT=obf[:, k * TILE:(k + 1) * TILE],
                                     rhs=fw[:, k, :],
                                     start=(k == 0), stop=(k == kt - 1))
                # ---- epilogue ----
                sl128 = slice(t * TILE, (t + 1) * TILE)
                slF = slice(t * F, (t + 1) * F)
                den = wk.tile([TILE, H], f32, tag="den", name=f"den{layer}_{t}")
                nc.vector.tensor_scalar(den[:, :], agg[:, F:F + H], 1e-9, None,
                                        op0=ALU.max)
                rec = wk.tile([TILE, H], f32, tag="rec", name=f"rec{layer}_{t}")
                nc.vector.reciprocal(rec[:, :], den[:, :])
                rst = ep.tile([TILE, F], f32, tag="rst", name=f"rst{layer}_{t}")
                av = agg[:, 0:F]
                a_b = bass.AP(av.tensor, av.offset, [av.ap[0], [HID, H], [1, HID]])
                rv = rec[:, 0:H]
                r_b = bass.AP(rv.tensor, rv.offset, [rv.ap[0], [1, H], [0, HID]])
                ov = rst[:, 0:F]
                o_b = bass.AP(ov.tensor, ov.offset, [ov.ap[0], [HID, H], [1, HID]])
                nc.vector.tensor_tensor(o_b, a_b, r_b, op=ALU.mult)
                if layer == 0:
                    xb_t = xbp.tile([TILE, F], f32, tag="xb", name=f"xb_{t}")
                    nc.scalar.dma_start(xb_t[:, :], xb_in[:, slF])
                    nc.vector.tensor_tensor(rst[:, :], rst[:, :], xb_t[:, :],
                                            op=ALU.add)
                    # ELU -> h
                    r1 = ep.tile([TILE, F], f32, tag="r1", name=f"r1_{t}")
                    nc.scalar.activation(r1[:, :], rst[:, :], AF.Relu)
                    r2 = ep.tile([TILE, F], f32, tag="r2", name=f"r2_{t}")
                    nc.scalar.activation(r2[:, :], rst[:, :], AF.Relu, scale=-1.0)
                    r3 = ep.tile([TILE, F], f32, tag="r3", name=f"r3_{t}")
                    nc.scalar.activation(r3[:, :], r2[:, :], AF.Exp, scale=-1.0)
                    nc.vector.scalar_tensor_tensor(h_sb[:, slF], r3[:, :], -1.0,
                                                   r1[:, :], ALU.add, ALU.add)
                    ptr = ps.tile([TILE, TILE], f32, tag="tr", bufs=1, name=f"tr_{t}")
                    nc.tensor.transpose(ptr[:, :], h_sb[:, slF], ident[:, :])
                    ht = ep.tile([TILE, TILE], f32, tag="ht", name=f"ht_{t}")
                    nc.vector.tensor_copy(ht[:, :], ptr[:, :])
                    # pipelined layer-2 node matmuls for this tile
                    pre_tile(ht[:, :], Wc2, 1, t)
                else:
                    nc.vector.tensor_tensor(rst[:, :], rst[:, :], h_sb[:, slF],
                                            op=ALU.add)
                    nc.vector.tensor_tensor(rst[:, :], rst[:, :], b2r[:, :],
                                            op=ALU.add)
                    m1 = ep.tile([TILE, OUTD], f32, tag="m1", name=f"m1_{t}")
                    nc.vector.tensor_tensor(m1[:, :], rst[:, 0:OUTD],
                                            rst[:, OUTD:2 * OUTD], op=ALU.add)
                    m2 = ep.tile([TILE, OUTD], f32, tag="m2", name=f"m2_{t}")
                    nc.vector.tensor_tensor(m2[:, :], rst[:, 2 * OUTD:3 * OUTD],
                                            rst[:, 3 * OUTD:4 * OUTD], op=ALU.add)
                    ot = ep.tile([TILE, OUTD], f32, tag="ot", name=f"ot_{t}")
                    nc.vector.tensor_tensor(ot[:, :], m1[:, :], m2[:, :], op=ALU.add)
                    of = ep.tile([TILE, OUTD], f32, tag="of", name=f"of_{t}")
                    nc.vector.tensor_scalar(of[:, :], ot[:, :], 0.25, None,
                                            op0=ALU.mult)
                    nc.sync.dma_start(out_ext[t * TILE:(t + 1) * TILE, :], of[:, :])

            def edge_phase(layer, post_b=None, skew=1):
                pend = []
                for t in range(t_tiles):
                    pend.append(stage_a(layer, t))
                    if t >= skew:
                        stage_b(layer, t - skew, *pend[t - skew])
                        if post_b and (t - skew) in post_b:
                            post_b[t - skew]()
                for t in range(t_tiles - skew, t_tiles):
                    stage_b(layer, t, *pend[t])
                    if post_b and t in post_b:
                        post_b[t]()

            # ================= layer 1 =================
            for nt in range(t_tiles):
                pre_tile(xT[:, nt * TILE:(nt + 1) * TILE], Wc1, 0, nt)
                if nt == TLO - 1:
                    allgather(0, 0)      # lo half ships while hi computes
            allgather(0, 1)
            # layer-2 fel halves ship as soon as their epilogues finish
            edge_phase(0, post_b={TLO - 1: lambda: allgather(1, 0),
                                  t_tiles - 1: lambda: allgather(1, 1)})
            # ================= layer 2 =================
            edge_phase(1)

    nc.compile()
    return nc


# ----------------------------------------------------------------------------
# Host entry point
# ----------------------------------------------------------------------------

def make_inputs(x, W1, al1, ar1, b1, W2, al2, ar2, b2, src, dst, n=N, p=P):
    KAs, KBs, per_core = prep_edges(np.asarray(src), np.asarray(dst), n=n, p=p)
    Wcat1 = prep_weights(np.asarray(W1, np.float32), np.asarray(al1, np.float32),
                         np.asarray(ar1, np.float32))
    Wcat2 = prep_weights(np.asarray(W2, np.float32), np.asarray(al2, np.float32),
                         np.asarray(ar2, np.float32))
    node_in = prep_node_inputs(np.asarray(x, np.float32), np.asarray(b1, np.float32),
                               n=n, p=p)
    b2r = np.tile(np.asarray(b2, np.float32)[None, :], (TILE, 1))
    ident = np.eye(TILE, dtype=np.float32)
    in_maps = []
    for c in range(p):
        xT, xb = node_in[c]
        pc = per_core[c]
        gB = pc["gB"] if pc["gB"].shape[1] else np.zeros((TILE, 16), np.int16)
        in_maps.append(dict(
            xT=xT, xb=xb, Wcat1=Wcat1, Wcat2=Wcat2, b2r=b2r, ident=ident,
            gA=pc["gA"], gB=gB, O8=pc["o8"], OT8=pc["ot8"]))
    return KAs, KBs, in_maps


def kernel(x, W1, al1, ar1, b1, W2, al2, ar2, b2, src, dst, **run_kwargs):
    from concourse.bass_utils import run_bass_kernel_spmd
    KAs, KBs, in_maps = make_inputs(x, W1, al1, ar1, b1, W2, al2, ar2, b2, src, dst)
    nc = build_gat(KAs, KBs)
    res = run_bass_kernel_spmd(nc, in_maps, core_ids=list(range(P)), **run_kwargs)
    out = np.concatenate([r["out"][:NLOC] for r in res.results], axis=0)
    if run_kwargs:
        return out.astype(np.float32), res
    return out.astype(np.float32)


# revision 52
# speedup vs baseline: 1.1055x; 1.0033x over previous
"""GAT (2-layer, 4-head) distributed Bass kernel for Trainium2, 8 NeuronCores.

Strategy (1D node partition, dst-owner edge routing), v2:
  - Core c owns nodes [c*NLOC, (c+1)*NLOC), padded to NLOCP = T*128.
  - Per layer: each core computes feat/el/er for its own nodes via PE matmuls
    (feat = x @ W, el = x @ (W@al), er = x @ (W@ar)), writes a bf16
    [NLOCP, 256] "fel" table ([feat(128) | el(4) | pad], 512 B rows) and
    AllGathers it across the 8 cores. er stays on-chip in SBUF (only the
    dst owner needs it).
  - The node table is split in two halves ("lo" = 31 node tiles, "hi" = 18)
    each AllGathered separately, so each table's int16 gather indices stay in
    range AND each AllGather can be kicked as soon as its half of the node
    epilogues is done (the layer-2 lo AllGather overlaps the layer-1 edge
    phase; only the smaller hi AllGather sits between the two edge phases).
  - Edges are grouped by destination owner, then by 128-row destination tile,
    then by source half; each (tile, half) stream is padded to whole 128-edge
    chunks, chunk counts maxed across cores so the SPMD IR is identical on
    all 8 cores.
  - The per-chunk one-hot matrices O[e, r] = (dst_row[e] == r) and their
    transposes are STATIC (host-known): they are precomputed on the host
    (O int8 -> cast to bf16 on DVE, OT fp8 used directly) and streamed in per
    tile as sequential DMA, replacing the per-chunk DVE is_equal build of v1
    (DVE-bound) and the er dma_gather of v1 (bandwidth-bound):
      * er matmul: er_ps[e, h] = sum_r OT[r, e] * er_tile[r, h]   (fp8 PE)
      * s = exp(leakyrelu(el[src] + er_ps))      (DVE + ACT)
      * featw = feat * s (head-broadcast); s into 4 denominator columns
      * PSUM accumulate: agg[r, :] += O.T @ featw  (numerator | denominator)
    Pad slots have all-zero one-hot columns, so they contribute nothing.
  - Per dst tile epilogue: rst = num/max(den,1e-9) + residual (+bias);
    layer 1 applies ELU, transposes h and immediately runs the layer-2
    node matmuls for that tile (pipelined pre-phase).
  - Each tile's gather chunks are balanced across the 4 SWDGE queues so
    their HBM transfers overlap.

Single-pass softmax: alpha = exp(e)/sum(exp(e)) == reference's
exp(e-emax)/sum(exp(e-emax)); logits are O(1) so no overflow.
"""

import os

# The NEFF backend (walrus_driver subprocess) schedules nondeterministically
# under hash randomization; some seeds cost ~12% HW time. Pin the seed the
# subprocess inherits so every compile lands on the fast schedule.
os.environ["PYTHONHASHSEED"] = "0"

import numpy as np
import ml_dtypes

# ---- problem constants (hardcoded; kernel.py must be self-contained) ----
N = 50000
E = 800000
P = 8
IN = 128
HID = 32
H = 4
F = H * HID          # 128, same for both layers
OUTD = 32
NEG = 0.2
TILE = 128

NLOC = N // P        # 6250
T = (NLOC + TILE - 1) // TILE          # 49
NLOCP = T * TILE     # 6272

ROWW = 256           # fel table row width in bf16 elems (512 B)
TLO = 31             # node tiles in the "lo" half-table (AllGathered early;
                     # 31 is the int16 max: 8*31*128 = 31744 < 32768)
THI = T - TLO        # 24 tiles in the "hi" half-table
NLO = TLO * TILE     # 3200 rows per core
NHI = THI * TILE     # 3072
BF16 = ml_dtypes.bfloat16


def _wrap16(idx):
    """[n] index list -> [128, n//16] int16, wrapped in 16 partitions and
    replicated across the 8 Q7 cores (dma_gather layout)."""
    a = np.asarray(idx).reshape(-1, 16).T
    return np.tile(a, (8, 1)).astype(np.int16)


# ----------------------------------------------------------------------------
# Host-side preprocessing
# ----------------------------------------------------------------------------

def prep_edges(src, dst, n=N, p=P):
    """Group edges by (dst owner, dst tile, src-half), pad each (core,tile,
    half) to common chunk counts KA_t/KB_t, and emit per-core index arrays.

    Returns (KAs, KBs, per_core): per_core[c] has
      gA   int16 [128, 8*sumKA]  wrapped fel-gather idxs into the lo table
      gB   int16 [128, 8*sumKB]  wrapped fel-gather idxs into the hi table
      oot  int8  [128, 2*sumK*128]  per-chunk one-hot blocks [O | OT] per tile
    """
    nloc = n // p
    t_tiles = (nloc + TILE - 1) // TILE

    owner = dst // nloc
    loc = dst - owner * nloc
    tl = loc // TILE
    row = loc - tl * TILE

    sowner = src // nloc
    sloc = src - sowner * nloc
    hb = (sloc >= NLO).astype(np.int64)           # 0 = lo table, 1 = hi
    pgid = np.where(hb == 0, sowner * NLO + sloc,
                    sowner * NHI + (sloc - NLO))

    order = np.lexsort((hb, tl, owner))
    owner_s = owner[order]
    tl_s = tl[order]
    hb_s = hb[order]
    row_s = row[order].astype(np.int64)
    pgid_s = pgid[order].astype(np.int64)

    counts = np.zeros((p, t_tiles, 2), dtype=np.int64)
    np.add.at(counts, (owner_s, tl_s, hb_s), 1)
    KAs = (-(-counts[:, :, 0] // TILE)).max(axis=0)
    KBs = (-(-counts[:, :, 1] // TILE)).max(axis=0)
    KAs = np.maximum(KAs, (KAs + KBs) == 0)       # ensure >=1 chunk per tile
    Ks = KAs + KBs
    off = np.concatenate([[0], np.cumsum(Ks)]).astype(int)
    offA = np.concatenate([[0], np.cumsum(KAs)]).astype(int)
    offB = np.concatenate([[0], np.cumsum(KBs)]).astype(int)
    sumK, sumKA, sumKB = int(off[-1]), int(offA[-1]), int(offB[-1])

    grp = (owner_s * t_tiles + tl_s) * 2 + hb_s
    gcnt = np.bincount(grp, minlength=p * t_tiles * 2)
    gstart = np.concatenate([[0], np.cumsum(gcnt)])
    within = np.arange(len(src)) - gstart[grp]
    k = within // TILE
    prt = within - k * TILE
    # chunk column in the full per-tile layout (A chunks first, then B)
    col = off[tl_s] + np.where(hb_s == 0, k, KAs[tl_s] + k)

    rng = np.arange(TILE, dtype=np.int64)
    per_core = []
    for c in range(p):
        m = owner_s == c
        dstrow = np.full((TILE, sumK), -1, dtype=np.int64)
        dstrow[prt[m], col[m]] = row_s[m]
        gfull = np.zeros((TILE, sumK), dtype=np.int64)
        gfull[prt[m], col[m]] = pgid_s[m]

        # one-hot blocks: oh[p, col, r] = (dstrow[p, col] == r)
        oh = (dstrow[:, :, None] == rng[None, None, :]).astype(np.int8)
        o8 = np.ascontiguousarray(oh.reshape(TILE, sumK * TILE))
        ohT = np.empty((TILE, sumK * TILE), dtype=ml_dtypes.float8_e4m3)
        for t in range(t_tiles):
            kt = int(Ks[t])
            o = int(off[t])
            blkT = oh[:, o:o + kt, :].transpose(2, 1, 0)  # [r, kt, p]
            ohT[:, o * TILE:(o + kt) * TILE] = \
                blkT.reshape(TILE, kt * TILE).astype(ml_dtypes.float8_e4m3)

        # flatten chunk cols into wrapped idx streams
        gA = np.zeros((TILE, 8 * sumKA), dtype=np.int16)
        gB = np.zeros((TILE, 8 * sumKB), dtype=np.int16)
        for t in range(t_tiles):
            ka, kb = int(KAs[t]), int(KBs[t])
            o, oa, ob = off[t], offA[t], offB[t]
            if ka:
                ia = gfull[:, o:o + ka].T.reshape(-1)          # i = k*128+p
                gA[:, 8 * oa:8 * (oa + ka)] = _wrap16(ia)
            if kb:
                ib = gfull[:, o + ka:o + ka + kb].T.reshape(-1)
                gB[:, 8 * ob:8 * (ob + kb)] = _wrap16(ib)
        per_core.append(dict(gA=gA, gB=gB, o8=o8, ot8=ohT))
    return [int(x) for x in KAs], [int(x) for x in KBs], per_core


def prep_weights(W, al, ar):
    """[W | W@al per head | W@ar per head] -> [in, F+2H] float32."""
    Wr = W.reshape(W.shape[0], H, -1)
    wal = np.einsum('ihd,hd->ih', Wr, al)
    war = np.einsum('ihd,hd->ih', Wr, ar)
    return np.concatenate([W, wal, war], axis=1).astype(np.float32)


def prep_node_inputs(x, b1, n=N, p=P):
    """Per-core xT ([IN, NLOCP], lhsT layout) and xb ([128, T*IN],
    tile-row-major residual layout, bias prefolded)."""
    nloc = n // p
    t_tiles = (nloc + TILE - 1) // TILE
    nlocp = t_tiles * TILE
    outs = []
    for c in range(p):
        xl = np.zeros((nlocp, x.shape[1]), dtype=np.float32)
        xl[:nloc] = x[c * nloc:(c + 1) * nloc]
        xT = np.ascontiguousarray(xl.T)
        xb = (xl + b1[None, :]).reshape(t_tiles, TILE, -1).transpose(1, 0, 2)
        xb = np.ascontiguousarray(xb.reshape(TILE, -1))
        outs.append((xT, xb))
    return outs


# ----------------------------------------------------------------------------
# Bass IR builder
# ----------------------------------------------------------------------------

def build_gat(KAs, KBs, n=N, p=P, in_dim=IN):
    import concourse.bass as bass
    import concourse.bacc as bacc
    import concourse.mybir as mybir
    import concourse.tile as tile

    f32 = mybir.dt.float32
    bf16 = mybir.dt.bfloat16
    i16 = mybir.dt.int16
    AF = mybir.ActivationFunctionType
    ALU = mybir.AluOpType

    nloc = n // p
    t_tiles = (nloc + TILE - 1) // TILE
    nlocp = t_tiles * TILE
    KAs = list(KAs)
    KBs = list(KBs)
    Ks = [a + b for a, b in zip(KAs, KBs)]
    off = np.concatenate([[0], np.cumsum(Ks)]).astype(int)
    offA = np.concatenate([[0], np.cumsum(KAs)]).astype(int)
    offB = np.concatenate([[0], np.cumsum(KBs)]).astype(int)
    sumK, sumKA, sumKB = int(off[-1]), int(offA[-1]), int(offB[-1])
    Kmax = max(Ks)
    rg = [list(range(p))]

    nc = bacc.Bacc("TRN2", target_bir_lowering=False, num_swdge_queues=4)

    # ---- I/O ----
    xT_in = nc.dram_tensor("xT", [in_dim, nlocp], f32, kind="ExternalInput")
    xb_in = nc.dram_tensor("xb", [TILE, t_tiles * in_dim], f32, kind="ExternalInput")
    W1_in = nc.dram_tensor("Wcat1", [in_dim, F + 2 * H], f32, kind="ExternalInput")
    W2_in = nc.dram_tensor("Wcat2", [F, F + 2 * H], f32, kind="ExternalInput")
    b2r_in = nc.dram_tensor("b2r", [TILE, F], f32, kind="ExternalInput")
    ident_in = nc.dram_tensor("ident", [TILE, TILE], f32, kind="ExternalInput")
    gA_in = nc.dram_tensor("gA", [TILE, 8 * sumKA], i16, kind="ExternalInput")
    gB_in = nc.dram_tensor("gB", [TILE, max(8 * sumKB, 16)], i16, kind="ExternalInput")
    o8_in = nc.dram_tensor("O8", [TILE, sumK * TILE], mybir.dt.int8,
                           kind="ExternalInput")
    ot8_in = nc.dram_tensor("OT8", [TILE, sumK * TILE], mybir.dt.float8e4,
                            kind="ExternalInput")
    out_ext = nc.dram_tensor("out", [nlocp, OUTD], f32, kind="ExternalOutput")

    # ---- internal DRAM ----
    fel_loc = [[nc.dram_tensor(f"fel_loc{i}{h}", [nn, ROWW], bf16)
                for h, nn in (("lo", NLO), ("hi", NHI))] for i in (1, 2)]
    fel_full = [[nc.dram_tensor(f"fel_full{i}{h}", [p * nn, ROWW], bf16,
                                addr_space="Shared")
                 for h, nn in (("lo", NLO), ("hi", NHI))] for i in (1, 2)]

    with tile.TileContext(nc) as tc:
        with tc.tile_pool(name="cst", bufs=1) as cst, \
             tc.tile_pool(name="big", bufs=1) as big, \
             tc.tile_pool(name="fe", bufs=4) as fep, \
             tc.tile_pool(name="oo", bufs=3) as oop, \
             tc.tile_pool(name="xbp", bufs=3) as xbp, \
             tc.tile_pool(name="wk", bufs=6) as wk, \
             tc.tile_pool(name="ep", bufs=3) as ep, \
             tc.tile_pool(name="ps", bufs=1, space="PSUM") as ps:

            xT = cst.sbuf_tile_from(xT_in.ap())
            Wc1 = cst.sbuf_tile_from(W1_in.ap())
            Wc2 = cst.sbuf_tile_from(W2_in.ap())
            b2r = cst.sbuf_tile_from(b2r_in.ap())
            ident = cst.sbuf_tile_from(ident_in.ap())
            gA = cst.sbuf_tile_from(gA_in.ap())
            gB = cst.sbuf_tile_from(gB_in.ap())

            h_sb = big.tile([TILE, t_tiles * F], f32)
            # per-node er for both layers, fp8 (rhs of the fp8 er matmul)
            er_sb = big.tile([TILE, 2 * t_tiles * H], mybir.dt.float8e4)

            def pre_tile(lhsT_ap, Wc, layer, nt):
                """node matmuls for one 128-node tile -> fel_loc rows + er_sb."""
                if nt < TLO:
                    dst_t = fel_loc[layer][0]
                    sl = slice(nt * TILE, (nt + 1) * TILE)
                else:
                    dst_t = fel_loc[layer][1]
                    sl = slice((nt - TLO) * TILE, (nt - TLO + 1) * TILE)
                pf = ps.tile([TILE, F], f32, tag="pf", bufs=2, name=f"pf{layer}_{nt}")
                nc.tensor.matmul(pf[:, :], lhsT=lhsT_ap, rhs=Wc[:, 0:F],
                                 start=True, stop=True)
                p8 = ps.tile([TILE, 2 * H], f32, tag="p8", bufs=1, name=f"p8{layer}_{nt}")
                nc.tensor.matmul(p8[:, :], lhsT=lhsT_ap, rhs=Wc[:, F:F + 2 * H],
                                 start=True, stop=True)
                fel = ep.tile([TILE, ROWW], bf16, tag="fel", name=f"fel{layer}_{nt}")
                nc.vector.tensor_copy(fel[:, 0:F], pf[:, :])
                nc.vector.tensor_copy(fel[:, F:F + H], p8[:, 0:H])
                nc.vector.tensor_copy(er_sb[:, (layer * t_tiles + nt) * H:
                                             (layer * t_tiles + nt + 1) * H],
                                      p8[:, H:2 * H])
                nc.sync.dma_start(dst_t[sl, 0:F + H], fel[:, 0:F + H])

            def allgather(layer, part):
                nc.gpsimd.collective_compute(
                    "AllGather", mybir.AluOpType.bypass, replica_groups=rg,
                    ins=[fel_loc[layer][part].ap().opt()],
                    outs=[fel_full[layer][part].ap().opt()])

            def stage_a(layer, t):
                """gathers + one-hot loads + er matmuls for dst tile t."""
                ka, kb = KAs[t], KBs[t]
                kt = ka + kb
                o0, oa, ob = int(off[t]), int(offA[t]), int(offB[t])
                o8t = oop.tile([TILE, kt * TILE], mybir.dt.int8, tag="o8",
                               bufs=2, padded_shape=[TILE, Kmax * TILE],
                               name=f"o8{layer}_{t}")
                nc.scalar.dma_start(o8t[:, :],
                                    o8_in[:, o0 * TILE:(o0 + kt) * TILE])
                obf = oop.tile([TILE, kt * TILE], bf16, tag="obf",
                               padded_shape=[TILE, Kmax * TILE],
                               name=f"obf{layer}_{t}")
                nc.vector.tensor_copy(obf[:, :], o8t[:, :])
                ot8 = oop.tile([TILE, kt * TILE], mybir.dt.float8e4, tag="ot8",
                               padded_shape=[TILE, Kmax * TILE],
                               name=f"ot8{layer}_{t}")
                nc.scalar.dma_start(ot8[:, :],
                                    ot8_in[:, o0 * TILE:(o0 + kt) * TILE])
                fe = fep.tile([TILE, kt, ROWW], bf16, tag="fe",
                              padded_shape=[TILE, Kmax, ROWW], name=f"fe{layer}_{t}")
                # balance the tile's kt chunks evenly across the 4 SWDGE
                # queues; a queue's range may span the lo/hi table boundary
                # (then it becomes two gather calls)
                bounds = [(i * kt + 2) // 4 for i in range(5)]
                for q in range(4):
                    s, e = bounds[q], bounds[q + 1]
                    if s < min(e, ka):
                        lo, hi = s, min(e, ka)
                        nc.gpsimd.dma_gather(
                            fe[:, lo:hi, :], fel_full[layer][0].ap(),
                            gA[:, 8 * (oa + lo):8 * (oa + hi)],
                            (hi - lo) * TILE, (hi - lo) * TILE, ROWW,
                            single_packet=True, queue_num=q)
                    if e > max(s, ka):
                        lo, hi = max(s, ka) - ka, e - ka
                        nc.gpsimd.dma_gather(
                            fe[:, ka + lo:ka + hi, :], fel_full[layer][1].ap(),
                            gB[:, 8 * (ob + lo):8 * (ob + hi)],
                            (hi - lo) * TILE, (hi - lo) * TILE, ROWW,
                            single_packet=True, queue_num=q)
                er_ps = ps.tile([TILE, Kmax * H], f32, tag="er", bufs=2,
                                name=f"erps{layer}_{t}")
                ert = er_sb[:, (layer * t_tiles + t) * H:(layer * t_tiles + t + 1) * H]
                for k in range(kt):
                    nc.tensor.matmul(er_ps[:, k * H:(k + 1) * H],
                                     lhsT=ot8[:, k * TILE:(k + 1) * TILE],
                                     rhs=ert, start=True, stop=True)
                return fe, obf, er_ps

            def stage_b(layer, t, fe, obf, er_ps):
                """SDDMM + softmax-weighted aggregation for dst tile t."""
                ka, kb = KAs[t], KBs[t]
                kt = ka + kb
                lg = wk.tile([TILE, kt * H], f32, tag="lg", bufs=3,
                             padded_shape=[TILE, Kmax * H], name=f"lg{layer}_{t}")
                nc.vector.tensor_tensor(lg[:, :], fe[:, :, F:F + H],
                                        er_ps[:, 0:kt * H], op=ALU.add)
                lr = wk.tile([TILE, kt * H], f32, tag="lr", bufs=3,
                             padded_shape=[TILE, Kmax * H], name=f"lr{layer}_{t}")
                nc.vector.scalar_tensor_tensor(lr[:, :], lg[:, :], NEG, lg[:, :],
                                               ALU.mult, ALU.max)
                fw = wk.tile([TILE, kt, F + H], bf16, tag="fw", bufs=3,
                             padded_shape=[TILE, Kmax, F + H], name=f"fw{layer}_{t}")
                nc.scalar.activation(fw[:, :, F:F + H], lr[:, :], AF.Exp)
                sv = fw[:, :, F:F + H]
                s_b = bass.AP(sv.tensor, sv.offset,
                              [sv.ap[0], [F + H, kt], [1, H], [0, HID]])
                nc.vector.tensor_tensor(fw[:, :, 0:F], fe[:, :, 0:F], s_b,
                                        op=ALU.mult)
                agg = ps.tile([TILE, F + H], f32, tag="agg", bufs=2,
                              name=f"agg{layer}_{t}")
                for k in range(kt):
                    nc.tensor.matmul(agg[:, :], lhsT=obf[:, k * TILE:(k + 1) * TILE],
                                     rhs=fw[:, k, :],
                                     start=(k == 0), stop=(k == kt - 1))
                # ---- epilogue ----
                sl128 = slice(t * TILE, (t + 1) * TILE)
                slF = slice(t * F, (t + 1) * F)
                den = wk.tile([TILE, H], f32, tag="den", name=f"den{layer}_{t}")
                nc.vector.tensor_scalar(den[:, :], agg[:, F:F + H], 1e-9, None,
                                        op0=ALU.max)
                rec = wk.tile([TILE, H], f32, tag="rec", name=f"rec{layer}_{t}")
                nc.vector.reciprocal(rec[:, :], den[:, :])
                rst = ep.tile([TILE, F], f32, tag="rst", name=f"rst{layer}_{t}")
                av = agg[:, 0:F]
                a_b = bass.AP(av.tensor, av.offset, [av.ap[0], [HID, H], [1, HID]])
                rv = rec[:, 0:H]
                r_b = bass.AP(rv.tensor, rv.offset, [rv.ap[0], [1, H], [0, HID]])
                ov = rst[:, 0:F]
                o_b = bass.AP(ov.tensor, ov.offset, [ov.ap[0], [HID, H], [1, HID]])
                nc.vector.tensor_tensor(o_b, a_b, r_b, op=ALU.mult)
                if layer == 0:
                    xb_t = xbp.tile([TILE, F], f32, tag="xb", name=f"xb_{t}")
                    nc.scalar.dma_start(xb_t[:, :], xb_in[:, slF])
                    nc.vector.tensor_tensor(rst[:, :], rst[:, :], xb_t[:, :],
                                            op=ALU.add)
                    # ELU -> h
                    r1 = ep.tile([TILE, F], f32, tag="r1", name=f"r1_{t}")
                    nc.scalar.activation(r1[:, :], rst[:, :], AF.Relu)
                    r2 = ep.tile([TILE, F], f32, tag="r2", name=f"r2_{t}")
                    nc.scalar.activation(r2[:, :], rst[:, :], AF.Relu, scale=-1.0)
                    r3 = ep.tile([TILE, F], f32, tag="r3", name=f"r3_{t}")
                    nc.scalar.activation(r3[:, :], r2[:, :], AF.Exp, scale=-1.0)
                    nc.vector.scalar_tensor_tensor(h_sb[:, slF], r3[:, :], -1.0,
                                                   r1[:, :], ALU.add, ALU.add)
                    ptr = ps.tile([TILE, TILE], f32, tag="tr", bufs=1, name=f"tr_{t}")
                    nc.tensor.transpose(ptr[:, :], h_sb[:, slF], ident[:, :])
                    ht = ep.tile([TILE, TILE], f32, tag="ht", name=f"ht_{t}")
                    nc.vector.tensor_copy(ht[:, :], ptr[:, :])
                    # pipelined layer-2 node matmuls for this tile
                    pre_tile(ht[:, :], Wc2, 1, t)
                else:
                    nc.vector.tensor_tensor(rst[:, :], rst[:, :], h_sb[:, slF],
                                            op=ALU.add)
                    nc.vector.tensor_tensor(rst[:, :], rst[:, :], b2r[:, :],
                                            op=ALU.add)
                    m1 = ep.tile([TILE, OUTD], f32, tag="m1", name=f"m1_{t}")
                    nc.vector.tensor_tensor(m1[:, :], rst[:, 0:OUTD],
                                            rst[:, OUTD:2 * OUTD], op=ALU.add)
                    m2 = ep.tile([TILE, OUTD], f32, tag="m2", name=f"m2_{t}")
                    nc.vector.tensor_tensor(m2[:, :], rst[:, 2 * OUTD:3 * OUTD],
                                            rst[:, 3 * OUTD:4 * OUTD], op=ALU.add)
                    ot = ep.tile([TILE, OUTD], f32, tag="ot", name=f"ot_{t}")
                    nc.vector.tensor_tensor(ot[:, :], m1[:, :], m2[:, :], op=ALU.add)
                    of = ep.tile([TILE, OUTD], f32, tag="of", name=f"of_{t}")
                    nc.vector.tensor_scalar(of[:, :], ot[:, :], 0.25, None,
                                            op0=ALU.mult)
                    nc.sync.dma_start(out_ext[t * TILE:(t + 1) * TILE, :], of[:, :])

            def edge_phase(layer, post_b=None, skew=1):
                pend = []
                for t in range(t_tiles):
                    pend.append(stage_a(layer, t))
                    if t >= skew:
                        stage_b(layer, t - skew, *pend[t - skew])
                        if post_b and (t - skew) in post_b:
                            post_b[t - skew]()
                for t in range(t_tiles - skew, t_tiles):
                    stage_b(layer, t, *pend[t])
                    if post_b and t in post_b:
                        post_b[t]()

            # ================= layer 1 =================
            for nt in range(t_tiles):
                pre_tile(xT[:, nt * TILE:(nt + 1) * TILE], Wc1, 0, nt)
                if nt == TLO - 1:
                    allgather(0, 0)      # lo half ships while hi computes
            allgather(0, 1)
            # layer-2 fel halves ship as soon as their epilogues finish
            edge_phase(0, post_b={TLO - 1: lambda: allgather(1, 0),
                                  t_tiles - 1: lambda: allgather(1, 1)})
            # ================= layer 2 =================
            edge_phase(1)

    nc.compile()
    return nc


# ----------------------------------------------------------------------------
# Host entry point
# ----------------------------------------------------------------------------

def make_inputs(x, W1, al1, ar1, b1, W2, al2, ar2, b2, src, dst, n=N, p=P):
    KAs, KBs, per_core = prep_edges(np.asarray(src), np.asarray(dst), n=n, p=p)
    Wcat1 = prep_weights(np.asarray(W1, np.float32), np.asarray(al1, np.float32),
                         np.asarray(ar1, np.float32))
    Wcat2 = prep_weights(np.asarray(W2, np.float32), np.asarray(al2, np.float32),
                         np.asarray(ar2, np.float32))
    node_in = prep_node_inputs(np.asarray(x, np.float32), np.asarray(b1, np.float32),
                               n=n, p=p)
    b2r = np.tile(np.asarray(b2, np.float32)[None, :], (TILE, 1))
    ident = np.eye(TILE, dtype=np.float32)
    in_maps = []
    for c in range(p):
        xT, xb = node_in[c]
        pc = per_core[c]
        gB = pc["gB"] if pc["gB"].shape[1] else np.zeros((TILE, 16), np.int16)
        in_maps.append(dict(
            xT=xT, xb=xb, Wcat1=Wcat1, Wcat2=Wcat2, b2r=b2r, ident=ident,
            gA=pc["gA"], gB=gB, O8=pc["o8"], OT8=pc["ot8"]))
    return KAs, KBs, in_maps


def kernel(x, W1, al1, ar1, b1, W2, al2, ar2, b2, src, dst, **run_kwargs):
    from concourse.bass_utils import run_bass_kernel_spmd
    KAs, KBs, in_maps = make_inputs(x, W1, al1, ar1, b1, W2, al2, ar2, b2, src, dst)
    nc = build_gat(KAs, KBs)
    res = run_bass_kernel_spmd(nc, in_maps, core_ids=list(range(P)), **run_kwargs)
    out = np.concatenate([r["out"][:NLOC] for r in res.results], axis=0)
    if run_kwargs:
        return out.astype(np.float32), res
    return out.astype(np.float32)


# revision 54
# speedup vs baseline: 1.1294x; 1.0216x over previous
"""GAT (2-layer, 4-head) distributed Bass kernel for Trainium2, 8 NeuronCores.

Strategy (1D node partition, dst-owner edge routing), v2:
  - Core c owns nodes [c*NLOC, (c+1)*NLOC), padded to NLOCP = T*128.
  - Per layer: each core computes feat/el/er for its own nodes via PE matmuls
    (feat = x @ W, el = x @ (W@al), er = x @ (W@ar)), writes a bf16
    [NLOCP, 256] "fel" table ([feat(128) | el(4) | pad], 512 B rows) and
    AllGathers it across the 8 cores. er stays on-chip in SBUF (only the
    dst owner needs it).
  - The node table is split in two halves ("lo" = 31 node tiles, "hi" = 18)
    each AllGathered separately, so each table's int16 gather indices stay in
    range AND each AllGather can be kicked as soon as its half of the node
    epilogues is done (the layer-2 lo AllGather overlaps the layer-1 edge
    phase; only the smaller hi AllGather sits between the two edge phases).
  - Edges are grouped by destination owner, then by 128-row destination tile,
    then by source half; each (tile, half) stream is padded to whole 128-edge
    chunks, chunk counts maxed across cores so the SPMD IR is identical on
    all 8 cores.
  - The per-chunk one-hot matrices O[e, r] = (dst_row[e] == r) and their
    transposes are STATIC (host-known): they are precomputed on the host
    (O int8 -> cast to bf16 on DVE, OT fp8 used directly) and streamed in per
    tile as sequential DMA, replacing the per-chunk DVE is_equal build of v1
    (DVE-bound) and the er dma_gather of v1 (bandwidth-bound):
      * er matmul: er_ps[e, h] = sum_r OT[r, e] * er_tile[r, h]   (fp8 PE)
      * s = exp(leakyrelu(el[src] + er_ps))      (DVE + ACT)
      * featw = feat * s (head-broadcast); s into 4 denominator columns
      * PSUM accumulate: agg[r, :] += O.T @ featw  (numerator | denominator)
    Pad slots have all-zero one-hot columns, so they contribute nothing.
  - Per dst tile epilogue: rst = num/max(den,1e-9) + residual (+bias);
    layer 1 applies ELU, transposes h and immediately runs the layer-2
    node matmuls for that tile (pipelined pre-phase).
  - Each tile's gather chunks are balanced across the 4 SWDGE queues so
    their HBM transfers overlap.

Single-pass softmax: alpha = exp(e)/sum(exp(e)) == reference's
exp(e-emax)/sum(exp(e-emax)); logits are O(1) so no overflow.
"""

import os

# The NEFF backend (walrus_driver subprocess) schedules nondeterministically
# under hash randomization; some seeds cost ~12% HW time. Pin the seed the
# subprocess inherits so every compile lands on the fast schedule.
os.environ["PYTHONHASHSEED"] = "0"

import numpy as np
import ml_dtypes

# ---- problem constants (hardcoded; kernel.py must be self-contained) ----
N = 50000
E = 800000
P = 8
IN = 128
HID = 32
H = 4
F = H * HID          # 128, same for both layers
OUTD = 32
NEG = 0.2
TILE = 128

NLOC = N // P        # 6250
T = (NLOC + TILE - 1) // TILE          # 49
NLOCP = T * TILE     # 6272

ROWW = 256           # fel table row width in bf16 elems (512 B)
TLO = 31             # node tiles in the "lo" half-table (AllGathered early;
                     # 31 is the int16 max: 8*31*128 = 31744 < 32768)
THI = T - TLO        # 24 tiles in the "hi" half-table
NLO = TLO * TILE     # 3200 rows per core
NHI = THI * TILE     # 3072
BF16 = ml_dtypes.bfloat16


def _wrap16(idx):
    """[n] index list -> [128, n//16] int16, wrapped in 16 partitions and
    replicated across the 8 Q7 cores (dma_gather layout)."""
    a = np.asarray(idx).reshape(-1, 16).T
    return np.tile(a, (8, 1)).astype(np.int16)


# ----------------------------------------------------------------------------
# Host-side preprocessing
# ----------------------------------------------------------------------------

def prep_edges(src, dst, n=N, p=P):
    """Group edges by (dst owner, dst tile, src-half), pad each (core,tile,
    half) to common chunk counts KA_t/KB_t, and emit per-core index arrays.

    Returns (KAs, KBs, per_core): per_core[c] has
      gA   int16 [128, 8*sumKA]  wrapped fel-gather idxs into the lo table
      gB   int16 [128, 8*sumKB]  wrapped fel-gather idxs into the hi table
      oot  int8  [128, 2*sumK*128]  per-chunk one-hot blocks [O | OT] per tile
    """
    nloc = n // p
    t_tiles = (nloc + TILE - 1) // TILE

    owner = dst // nloc
    loc = dst - owner * nloc
    tl = loc // TILE
    row = loc - tl * TILE

    sowner = src // nloc
    sloc = src - sowner * nloc
    hb = (sloc >= NLO).astype(np.int64)           # 0 = lo table, 1 = hi
    pgid = np.where(hb == 0, sowner * NLO + sloc,
                    sowner * NHI + (sloc - NLO))

    order = np.lexsort((hb, tl, owner))
    owner_s = owner[order]
    tl_s = tl[order]
    hb_s = hb[order]
    row_s = row[order].astype(np.int64)
    pgid_s = pgid[order].astype(np.int64)

    counts = np.zeros((p, t_tiles, 2), dtype=np.int64)
    np.add.at(counts, (owner_s, tl_s, hb_s), 1)
    KAs = (-(-counts[:, :, 0] // TILE)).max(axis=0)
    KBs = (-(-counts[:, :, 1] // TILE)).max(axis=0)
    KAs = np.maximum(KAs, (KAs + KBs) == 0)       # ensure >=1 chunk per tile
    Ks = KAs + KBs
    off = np.concatenate([[0], np.cumsum(Ks)]).astype(int)
    offA = np.concatenate([[0], np.cumsum(KAs)]).astype(int)
    offB = np.concatenate([[0], np.cumsum(KBs)]).astype(int)
    sumK, sumKA, sumKB = int(off[-1]), int(offA[-1]), int(offB[-1])

    grp = (owner_s * t_tiles + tl_s) * 2 + hb_s
    gcnt = np.bincount(grp, minlength=p * t_tiles * 2)
    gstart = np.concatenate([[0], np.cumsum(gcnt)])
    within = np.arange(len(src)) - gstart[grp]
    k = within // TILE
    prt = within - k * TILE
    # chunk column in the full per-tile layout (A chunks first, then B)
    col = off[tl_s] + np.where(hb_s == 0, k, KAs[tl_s] + k)

    rng = np.arange(TILE, dtype=np.int64)
    per_core = []
    for c in range(p):
        m = owner_s == c
        dstrow = np.full((TILE, sumK), -1, dtype=np.int64)
        dstrow[prt[m], col[m]] = row_s[m]
        gfull = np.zeros((TILE, sumK), dtype=np.int64)
        gfull[prt[m], col[m]] = pgid_s[m]

        # one-hot blocks: oh[p, col, r] = (dstrow[p, col] == r)
        oh = (dstrow[:, :, None] == rng[None, None, :]).astype(np.int8)
        o8 = np.ascontiguousarray(oh.reshape(TILE, sumK * TILE))
        ohT = np.empty((TILE, sumK * TILE), dtype=ml_dtypes.float8_e4m3)
        for t in range(t_tiles):
            kt = int(Ks[t])
            o = int(off[t])
            blkT = oh[:, o:o + kt, :].transpose(2, 1, 0)  # [r, kt, p]
            ohT[:, o * TILE:(o + kt) * TILE] = \
                blkT.reshape(TILE, kt * TILE).astype(ml_dtypes.float8_e4m3)

        # flatten chunk cols into wrapped idx streams
        gA = np.zeros((TILE, 8 * sumKA), dtype=np.int16)
        gB = np.zeros((TILE, 8 * sumKB), dtype=np.int16)
        for t in range(t_tiles):
            ka, kb = int(KAs[t]), int(KBs[t])
            o, oa, ob = off[t], offA[t], offB[t]
            if ka:
                ia = gfull[:, o:o + ka].T.reshape(-1)          # i = k*128+p
                gA[:, 8 * oa:8 * (oa + ka)] = _wrap16(ia)
            if kb:
                ib = gfull[:, o + ka:o + ka + kb].T.reshape(-1)
                gB[:, 8 * ob:8 * (ob + kb)] = _wrap16(ib)
        per_core.append(dict(gA=gA, gB=gB, o8=o8, ot8=ohT))
    return [int(x) for x in KAs], [int(x) for x in KBs], per_core


def prep_weights(W, al, ar):
    """[W | W@al per head | W@ar per head] -> [in, F+2H] float32."""
    Wr = W.reshape(W.shape[0], H, -1)
    wal = np.einsum('ihd,hd->ih', Wr, al)
    war = np.einsum('ihd,hd->ih', Wr, ar)
    return np.concatenate([W, wal, war], axis=1).astype(np.float32)


def prep_node_inputs(x, b1, n=N, p=P):
    """Per-core xT ([IN, NLOCP], lhsT layout) and xb ([128, T*IN],
    tile-row-major residual layout, bias prefolded)."""
    nloc = n // p
    t_tiles = (nloc + TILE - 1) // TILE
    nlocp = t_tiles * TILE
    outs = []
    for c in range(p):
        xl = np.zeros((nlocp, x.shape[1]), dtype=np.float32)
        xl[:nloc] = x[c * nloc:(c + 1) * nloc]
        xT = np.ascontiguousarray(xl.T)
        xb = (xl + b1[None, :]).reshape(t_tiles, TILE, -1).transpose(1, 0, 2)
        xb = np.ascontiguousarray(xb.reshape(TILE, -1))
        outs.append((xT, xb))
    return outs


# ----------------------------------------------------------------------------
# Bass IR builder
# ----------------------------------------------------------------------------

def build_gat(KAs, KBs, n=N, p=P, in_dim=IN):
    import concourse.bass as bass
    import concourse.bacc as bacc
    import concourse.mybir as mybir
    import concourse.tile as tile

    f32 = mybir.dt.float32
    bf16 = mybir.dt.bfloat16
    i16 = mybir.dt.int16
    AF = mybir.ActivationFunctionType
    ALU = mybir.AluOpType

    nloc = n // p
    t_tiles = (nloc + TILE - 1) // TILE
    nlocp = t_tiles * TILE
    KAs = list(KAs)
    KBs = list(KBs)
    Ks = [a + b for a, b in zip(KAs, KBs)]
    off = np.concatenate([[0], np.cumsum(Ks)]).astype(int)
    offA = np.concatenate([[0], np.cumsum(KAs)]).astype(int)
    offB = np.concatenate([[0], np.cumsum(KBs)]).astype(int)
    sumK, sumKA, sumKB = int(off[-1]), int(offA[-1]), int(offB[-1])
    Kmax = max(Ks)
    rg = [list(range(p))]

    nc = bacc.Bacc("TRN2", target_bir_lowering=False, num_swdge_queues=4)

    # ---- I/O ----
    xT_in = nc.dram_tensor("xT", [in_dim, nlocp], bf16, kind="ExternalInput")
    xb_in = nc.dram_tensor("xb", [TILE, t_tiles * in_dim], f32, kind="ExternalInput")
    W1_in = nc.dram_tensor("Wcat1", [in_dim, F + 2 * H], bf16, kind="ExternalInput")
    W2_in = nc.dram_tensor("Wcat2", [F, F + 2 * H], f32, kind="ExternalInput")
    b2r_in = nc.dram_tensor("b2r", [TILE, F], f32, kind="ExternalInput")
    ident_in = nc.dram_tensor("ident", [TILE, TILE], f32, kind="ExternalInput")
    gA_in = nc.dram_tensor("gA", [TILE, 8 * sumKA], i16, kind="ExternalInput")
    gB_in = nc.dram_tensor("gB", [TILE, max(8 * sumKB, 16)], i16, kind="ExternalInput")
    o8_in = nc.dram_tensor("O8", [TILE, sumK * TILE], mybir.dt.int8,
                           kind="ExternalInput")
    ot8_in = nc.dram_tensor("OT8", [TILE, sumK * TILE], mybir.dt.float8e4,
                            kind="ExternalInput")
    out_ext = nc.dram_tensor("out", [nlocp, OUTD], f32, kind="ExternalOutput")

    # ---- internal DRAM ----
    fel_loc = [[nc.dram_tensor(f"fel_loc{i}{h}", [nn, ROWW], bf16)
                for h, nn in (("lo", NLO), ("hi", NHI))] for i in (1, 2)]
    fel_full = [[nc.dram_tensor(f"fel_full{i}{h}", [p * nn, ROWW], bf16,
                                addr_space="Shared")
                 for h, nn in (("lo", NLO), ("hi", NHI))] for i in (1, 2)]

    with tile.TileContext(nc) as tc:
        with tc.tile_pool(name="cst", bufs=1) as cst, \
             tc.tile_pool(name="big", bufs=1) as big, \
             tc.tile_pool(name="fe", bufs=4) as fep, \
             tc.tile_pool(name="oo", bufs=3) as oop, \
             tc.tile_pool(name="xbp", bufs=3) as xbp, \
             tc.tile_pool(name="wk", bufs=6) as wk, \
             tc.tile_pool(name="ep", bufs=3) as ep, \
             tc.tile_pool(name="ps", bufs=1, space="PSUM") as ps:

            xT = cst.sbuf_tile_from(xT_in.ap())
            Wc1 = cst.sbuf_tile_from(W1_in.ap())
            Wc2 = cst.sbuf_tile_from(W2_in.ap())
            b2r = cst.sbuf_tile_from(b2r_in.ap())
            ident = cst.sbuf_tile_from(ident_in.ap())
            gA = cst.sbuf_tile_from(gA_in.ap())
            gB = cst.sbuf_tile_from(gB_in.ap())

            h_sb = big.tile([TILE, t_tiles * F], f32)
            # per-node er for both layers, fp8 (rhs of the fp8 er matmul)
            er_sb = big.tile([TILE, 2 * t_tiles * H], mybir.dt.float8e4)

            def pre_tile(lhsT_ap, Wc, layer, nt):
                """node matmuls for one 128-node tile -> fel_loc rows + er_sb."""
                if nt < TLO:
                    dst_t = fel_loc[layer][0]
                    sl = slice(nt * TILE, (nt + 1) * TILE)
                else:
                    dst_t = fel_loc[layer][1]
                    sl = slice((nt - TLO) * TILE, (nt - TLO + 1) * TILE)
                pf = ps.tile([TILE, F], f32, tag="pf", bufs=2, name=f"pf{layer}_{nt}")
                nc.tensor.matmul(pf[:, :], lhsT=lhsT_ap, rhs=Wc[:, 0:F],
                                 start=True, stop=True)
                p8 = ps.tile([TILE, 2 * H], f32, tag="p8", bufs=1, name=f"p8{layer}_{nt}")
                nc.tensor.matmul(p8[:, :], lhsT=lhsT_ap, rhs=Wc[:, F:F + 2 * H],
                                 start=True, stop=True)
                fel = ep.tile([TILE, ROWW], bf16, tag="fel", name=f"fel{layer}_{nt}")
                nc.vector.tensor_copy(fel[:, 0:F], pf[:, :])
                nc.vector.tensor_copy(fel[:, F:F + H], p8[:, 0:H])
                nc.vector.tensor_copy(er_sb[:, (layer * t_tiles + nt) * H:
                                             (layer * t_tiles + nt + 1) * H],
                                      p8[:, H:2 * H])
                nc.sync.dma_start(dst_t[sl, 0:F + H], fel[:, 0:F + H])

            def allgather(layer, part):
                nc.gpsimd.collective_compute(
                    "AllGather", mybir.AluOpType.bypass, replica_groups=rg,
                    ins=[fel_loc[layer][part].ap().opt()],
                    outs=[fel_full[layer][part].ap().opt()])

            def stage_a(layer, t):
                """gathers + one-hot loads + er matmuls for dst tile t."""
                ka, kb = KAs[t], KBs[t]
                kt = ka + kb
                o0, oa, ob = int(off[t]), int(offA[t]), int(offB[t])
                o8t = oop.tile([TILE, kt * TILE], mybir.dt.int8, tag="o8",
                               bufs=2, padded_shape=[TILE, Kmax * TILE],
                               name=f"o8{layer}_{t}")
                nc.scalar.dma_start(o8t[:, :],
                                    o8_in[:, o0 * TILE:(o0 + kt) * TILE])
                obf = oop.tile([TILE, kt * TILE], bf16, tag="obf",
                               padded_shape=[TILE, Kmax * TILE],
                               name=f"obf{layer}_{t}")
                nc.vector.tensor_copy(obf[:, :], o8t[:, :])
                ot8 = oop.tile([TILE, kt * TILE], mybir.dt.float8e4, tag="ot8",
                               padded_shape=[TILE, Kmax * TILE],
                               name=f"ot8{layer}_{t}")
                nc.scalar.dma_start(ot8[:, :],
                                    ot8_in[:, o0 * TILE:(o0 + kt) * TILE])
                fe = fep.tile([TILE, kt, ROWW], bf16, tag="fe",
                              padded_shape=[TILE, Kmax, ROWW], name=f"fe{layer}_{t}")
                # balance the tile's kt chunks evenly across the 4 SWDGE
                # queues; a queue's range may span the lo/hi table boundary
                # (then it becomes two gather calls)
                bounds = [(i * kt + 2) // 4 for i in range(5)]
                for q in range(4):
                    s, e = bounds[q], bounds[q + 1]
                    if s < min(e, ka):
                        lo, hi = s, min(e, ka)
                        nc.gpsimd.dma_gather(
                            fe[:, lo:hi, :], fel_full[layer][0].ap(),
                            gA[:, 8 * (oa + lo):8 * (oa + hi)],
                            (hi - lo) * TILE, (hi - lo) * TILE, ROWW,
                            single_packet=True, queue_num=q)
                    if e > max(s, ka):
                        lo, hi = max(s, ka) - ka, e - ka
                        nc.gpsimd.dma_gather(
                            fe[:, ka + lo:ka + hi, :], fel_full[layer][1].ap(),
                            gB[:, 8 * (ob + lo):8 * (ob + hi)],
                            (hi - lo) * TILE, (hi - lo) * TILE, ROWW,
                            single_packet=True, queue_num=q)
                er_ps = ps.tile([TILE, Kmax * H], f32, tag="er", bufs=2,
                                name=f"erps{layer}_{t}")
                ert = er_sb[:, (layer * t_tiles + t) * H:(layer * t_tiles + t + 1) * H]
                for k in range(kt):
                    nc.tensor.matmul(er_ps[:, k * H:(k + 1) * H],
                                     lhsT=ot8[:, k * TILE:(k + 1) * TILE],
                                     rhs=ert, start=True, stop=True)
                return fe, obf, er_ps

            def stage_b(layer, t, fe, obf, er_ps):
                """SDDMM + softmax-weighted aggregation for dst tile t."""
                ka, kb = KAs[t], KBs[t]
                kt = ka + kb
                lg = wk.tile([TILE, kt * H], f32, tag="lg", bufs=3,
                             padded_shape=[TILE, Kmax * H], name=f"lg{layer}_{t}")
                nc.vector.tensor_tensor(lg[:, :], fe[:, :, F:F + H],
                                        er_ps[:, 0:kt * H], op=ALU.add)
                lr = wk.tile([TILE, kt * H], f32, tag="lr", bufs=3,
                             padded_shape=[TILE, Kmax * H], name=f"lr{layer}_{t}")
                nc.vector.scalar_tensor_tensor(lr[:, :], lg[:, :], NEG, lg[:, :],
                                               ALU.mult, ALU.max)
                fw = wk.tile([TILE, kt, F + H], bf16, tag="fw", bufs=3,
                             padded_shape=[TILE, Kmax, F + H], name=f"fw{layer}_{t}")
                nc.scalar.activation(fw[:, :, F:F + H], lr[:, :], AF.Exp)
                sv = fw[:, :, F:F + H]
                s_b = bass.AP(sv.tensor, sv.offset,
                              [sv.ap[0], [F + H, kt], [1, H], [0, HID]])
                nc.vector.tensor_tensor(fw[:, :, 0:F], fe[:, :, 0:F], s_b,
                                        op=ALU.mult)
                agg = ps.tile([TILE, F + H], f32, tag="agg", bufs=2,
                              name=f"agg{layer}_{t}")
                for k in range(kt):
                    nc.tensor.matmul(agg[:, :], lhsT=obf[:, k * TILE:(k + 1) * TILE],
                                     rhs=fw[:, k, :],
                                     start=(k == 0), stop=(k == kt - 1))
                # ---- epilogue ----
                sl128 = slice(t * TILE, (t + 1) * TILE)
                slF = slice(t * F, (t + 1) * F)
                den = wk.tile([TILE, H], f32, tag="den", name=f"den{layer}_{t}")
                nc.vector.tensor_scalar(den[:, :], agg[:, F:F + H], 1e-9, None,
                                        op0=ALU.max)
                rec = wk.tile([TILE, H], f32, tag="rec", name=f"rec{layer}_{t}")
                nc.vector.reciprocal(rec[:, :], den[:, :])
                rst = ep.tile([TILE, F], f32, tag="rst", name=f"rst{layer}_{t}")
                av = agg[:, 0:F]
                a_b = bass.AP(av.tensor, av.offset, [av.ap[0], [HID, H], [1, HID]])
                rv = rec[:, 0:H]
                r_b = bass.AP(rv.tensor, rv.offset, [rv.ap[0], [1, H], [0, HID]])
                ov = rst[:, 0:F]
                o_b = bass.AP(ov.tensor, ov.offset, [ov.ap[0], [HID, H], [1, HID]])
                nc.vector.tensor_tensor(o_b, a_b, r_b, op=ALU.mult)
                if layer == 0:
                    xb_t = xbp.tile([TILE, F], f32, tag="xb", name=f"xb_{t}")
                    nc.scalar.dma_start(xb_t[:, :], xb_in[:, slF])
                    nc.vector.tensor_tensor(rst[:, :], rst[:, :], xb_t[:, :],
                                            op=ALU.add)
                    # ELU -> h
                    r1 = ep.tile([TILE, F], f32, tag="r1", name=f"r1_{t}")
                    nc.scalar.activation(r1[:, :], rst[:, :], AF.Relu)
                    r2 = ep.tile([TILE, F], f32, tag="r2", name=f"r2_{t}")
                    nc.scalar.activation(r2[:, :], rst[:, :], AF.Relu, scale=-1.0)
                    r3 = ep.tile([TILE, F], f32, tag="r3", name=f"r3_{t}")
                    nc.scalar.activation(r3[:, :], r2[:, :], AF.Exp, scale=-1.0)
                    nc.vector.scalar_tensor_tensor(h_sb[:, slF], r3[:, :], -1.0,
                                                   r1[:, :], ALU.add, ALU.add)
                    ptr = ps.tile([TILE, TILE], f32, tag="tr", bufs=1, name=f"tr_{t}")
                    nc.tensor.transpose(ptr[:, :], h_sb[:, slF], ident[:, :])
                    ht = ep.tile([TILE, TILE], f32, tag="ht", name=f"ht_{t}")
                    nc.vector.tensor_copy(ht[:, :], ptr[:, :])
                    # pipelined layer-2 node matmuls for this tile
                    pre_tile(ht[:, :], Wc2, 1, t)
                else:
                    nc.vector.tensor_tensor(rst[:, :], rst[:, :], h_sb[:, slF],
                                            op=ALU.add)
                    nc.vector.tensor_tensor(rst[:, :], rst[:, :], b2r[:, :],
                                            op=ALU.add)
                    m1 = ep.tile([TILE, OUTD], f32, tag="m1", name=f"m1_{t}")
                    nc.vector.tensor_tensor(m1[:, :], rst[:, 0:OUTD],
                                            rst[:, OUTD:2 * OUTD], op=ALU.add)
                    m2 = ep.tile([TILE, OUTD], f32, tag="m2", name=f"m2_{t}")
                    nc.vector.tensor_tensor(m2[:, :], rst[:, 2 * OUTD:3 * OUTD],
                                            rst[:, 3 * OUTD:4 * OUTD], op=ALU.add)
                    ot = ep.tile([TILE, OUTD], f32, tag="ot", name=f"ot_{t}")
                    nc.vector.tensor_tensor(ot[:, :], m1[:, :], m2[:, :], op=ALU.add)
                    of = ep.tile([TILE, OUTD], f32, tag="of", name=f"of_{t}")
                    nc.vector.tensor_scalar(of[:, :], ot[:, :], 0.25, None,
                                            op0=ALU.mult)
                    nc.sync.dma_start(out_ext[t * TILE:(t + 1) * TILE, :], of[:, :])

            def edge_phase(layer, post_b=None, skew=1):
                pend = []
                for t in range(t_tiles):
                    pend.append(stage_a(layer, t))
                    if t >= skew:
                        stage_b(layer, t - skew, *pend[t - skew])
                        if post_b and (t - skew) in post_b:
                            post_b[t - skew]()
                for t in range(t_tiles - skew, t_tiles):
                    stage_b(layer, t, *pend[t])
                    if post_b and t in post_b:
                        post_b[t]()

            # ================= layer 1 =================
            for nt in range(t_tiles):
                pre_tile(xT[:, nt * TILE:(nt + 1) * TILE], Wc1, 0, nt)
                if nt == TLO - 1:
                    allgather(0, 0)      # lo half ships while hi computes
            allgather(0, 1)
            # layer-2 fel halves ship as soon as their epilogues finish
            edge_phase(0, post_b={TLO - 1: lambda: allgather(1, 0),
                                  t_tiles - 1: lambda: allgather(1, 1)})
            # ================= layer 2 =================
            edge_phase(1)

    nc.compile()
    return nc


# ----------------------------------------------------------------------------
# Host entry point
# ----------------------------------------------------------------------------

def make_inputs(x, W1, al1, ar1, b1, W2, al2, ar2, b2, src, dst, n=N, p=P):
    KAs, KBs, per_core = prep_edges(np.asarray(src), np.asarray(dst), n=n, p=p)
    Wcat1 = prep_weights(np.asarray(W1, np.float32), np.asarray(al1, np.float32),
                         np.asarray(ar1, np.float32))
    Wcat2 = prep_weights(np.asarray(W2, np.float32), np.asarray(al2, np.float32),
                         np.asarray(ar2, np.float32))
    node_in = prep_node_inputs(np.asarray(x, np.float32), np.asarray(b1, np.float32),
                               n=n, p=p)
    b2r = np.tile(np.asarray(b2, np.float32)[None, :], (TILE, 1))
    ident = np.eye(TILE, dtype=np.float32)
    in_maps = []
    for c in range(p):
        xT, xb = node_in[c]
        pc = per_core[c]
        gB = pc["gB"] if pc["gB"].shape[1] else np.zeros((TILE, 16), np.int16)
        in_maps.append(dict(
            xT=xT.astype(BF16), xb=xb, Wcat1=Wcat1.astype(BF16), Wcat2=Wcat2,
            b2r=b2r, ident=ident,
            gA=pc["gA"], gB=gB, O8=pc["o8"], OT8=pc["ot8"]))
    return KAs, KBs, in_maps


def kernel(x, W1, al1, ar1, b1, W2, al2, ar2, b2, src, dst, **run_kwargs):
    from concourse.bass_utils import run_bass_kernel_spmd
    KAs, KBs, in_maps = make_inputs(x, W1, al1, ar1, b1, W2, al2, ar2, b2, src, dst)
    nc = build_gat(KAs, KBs)
    res = run_bass_kernel_spmd(nc, in_maps, core_ids=list(range(P)), **run_kwargs)
    out = np.concatenate([r["out"][:NLOC] for r in res.results], axis=0)
    if run_kwargs:
        return out.astype(np.float32), res
    return out.astype(np.float32)
